# revision 15
# baseline (speedup 1.0000x reference)
"""Trainium2 Bass kernel for nn_ContinuousGenHyperConnectionsStrang.

Contract: kernel(**inputs) takes FULL unsharded inputs (as in
reference.setup_inputs()) and returns the FULL [4096, 4, 2048] f32 output.

Strategy (8 cores, data-parallel over tokens, 512 tokens/core):
  - Front: x loaded token-major (4 tiles resident); RMS stats + h_all =
    xn @ W_all.T in f32r (PE transposes, 256-wide token pairs). Only the
    read-in gates ri are computed before the MLP; the rest of the per-token
    generator scalars (4x4 Cayley closed form) runs overlapped with MLP1.
  - branch = sum_n ri_n x_n quantized to fp8-e4m3 hi+lo pairs (quant error
    compensated); W1/W2 pre-scaled x256 and cast to fp8 on the host.
  - MLP entirely fp8 DoubleRow (2 k-planes per matmul instruction): MLP1
    computes branch-hi and branch-lo products; g1 = gelu re-quantized hi+lo;
    MLP2 likewise. Residual error is weight quantization only (~1.6e-2 rel).
  - Stream mix out_i = sum_j Phi_ij x_j computed in bf16 (2x DVE rate) from
    a host-cast bf16 copy of x, overlapped with MLP; the tail is fused into
    MLP2 half 1 per 512-wide embed chunk (transpose y from PSUM, STT with
    wo_i straight out of PSUM, chunked output DMAs).
"""
import numpy as np
import ml_dtypes

import concourse.bass as bass
import concourse.bacc as bacc
import concourse.mybir as mybir
import concourse.tile as tile
from concourse.bass_utils import run_bass_kernel_spmd
from concourse.masks import make_identity
from contextlib import ExitStack

F32 = mybir.dt.float32
F32R = mybir.dt.float32r
BF16 = mybir.dt.bfloat16
F8 = mybir.dt.float8e4
AF = mybir.ActivationFunctionType
OP = mybir.AluOpType
DR = mybir.MatmulPerfMode.DoubleRow

NCORES = 8
B_FULL = 4096
TPC = B_FULL // NCORES          # 512 tokens per core
TT = TPC // 128                 # 4 token tiles
NS = 4                          # streams (N)
ED = 2048                       # EMBED / BLOCK
IN_DIM = NS * ED                # 8192
FD = 8192                       # DFF
KT_IN = IN_DIM // 128           # 64 k-tiles over input dim
KT_ED = ED // 128               # 16 k-tiles over embed
KP_ED = KT_ED // 2              # 8 k-pairs over embed
KT_FDH = 32                     # dff k-tiles per half
KP_FDH = KT_FDH // 2            # 16 k-pairs per dff half
WSCALE = 256.0
DT_MIN, DT_MAX = 1e-3, 1.0
DT_RANGE = DT_MAX - DT_MIN
EPS = 1.1920929e-7

PAIRS = [(0, 1), (0, 2), (0, 3), (2, 3), (1, 3), (1, 2)]
PIDX = {p: k for k, p in enumerate(PAIRS)}
U_ROWS = [4 * i + j for (i, j) in PAIRS]
L_ROWS = [4 * j + i for (i, j) in PAIRS]


def build_nc():
    nc = bacc.Bacc()
    x_d = nc.declare_dram_parameter("x", [TPC, NS, ED], F32, isOutput=False)
    xh_d = nc.declare_dram_parameter("xh", [TPC, NS, ED], BF16, isOutput=False)
    wall_d = nc.declare_dram_parameter("wall", [IN_DIM, 32], F32R, isOutput=False)
    # w1: [mgrp(16), p(128), kp(8) x ktl(2) x m(512)] - one DMA per mgrp
    w1_d = nc.declare_dram_parameter("w1", [16, 128, KP_ED * 2 * 512], F8,
                                     isOutput=False)
    # w2: [half(2), mgrp2(4), kph(2), p(128), kp(8) x ktl(2) x m(512)]
    w2_d = nc.declare_dram_parameter("w2", [2, 4, 2, 128, 8 * 2 * 512], F8,
                                     isOutput=False)
    cvec_d = nc.declare_dram_parameter("cvec", [1, 64], F32, isOutput=False)
    out_d = nc.declare_dram_parameter("out", [TPC, NS, ED], F32, isOutput=True)

    with tile.TileContext(nc) as tc, ExitStack() as S0:
        const = S0.enter_context(tc.tile_pool(name="const", bufs=1))
        scal = S0.enter_context(tc.tile_pool(name="scal", bufs=1))
        soupp = S0.enter_context(tc.tile_pool(name="soupp", bufs=4))
        brp = S0.enter_context(tc.tile_pool(name="brp", bufs=1))

        ident = const.tile([128, 128], F32)
        make_identity(nc, ident[:])
        ones1 = const.tile([1, 128], F32)
        nc.gpsimd.memset(ones1[:], 1.0)
        cvec_sb = const.tile([1, 64], F32)
        nc.sync.dma_start(cvec_sb[:], cvec_d[:])

        # persistent per-token scalar outputs
        C = const.tile([128, 64], F32)
        rms = scal.tile([128, TT], F32)
        hscal = scal.tile([128, 32 * TT], F32)
        ri4 = [scal.tile([128, 4], F32, tag=f"ri{t}", name=f"ri{t}") for t in range(TT)]
        wo2 = [scal.tile([128, 4], F32, tag=f"wo{t}", name=f"wo{t}") for t in range(TT)]
        PhiP = [scal.tile([128, 6], F32, tag=f"pp{t}", name=f"pp{t}") for t in range(TT)]
        PhiM = [scal.tile([128, 6], F32, tag=f"pm{t}", name=f"pm{t}") for t in range(TT)]
        PhiD = [scal.tile([128, 4], F32, tag=f"pd{t}", name=f"pd{t}") for t in range(TT)]

        def phi_ap(t, i, j):
            if i == j:
                return PhiD[t][:, i:i + 1]
            if (i, j) in PIDX:
                return PhiP[t][:, PIDX[(i, j)]:PIDX[(i, j)] + 1]
            return PhiM[t][:, PIDX[(j, i)]:PIDX[(j, i)] + 1]

        # branchT fp8 hi+lo: [128(k), kt(16) x role(2) x tok(512)] = 16KB
        brT = brp.tile([128, KT_ED * 2 * TPC], F8)
        brT_v = brT[:].rearrange("p (kt r n) -> p kt r n", kt=KT_ED, r=2)

        # ================= FRONT =================
        with ExitStack() as SA:
            xtokp = SA.enter_context(tc.tile_pool(name="xtok", bufs=1))
            wallp = SA.enter_context(tc.tile_pool(name="wallp", bufs=1))
            xtp = SA.enter_context(tc.tile_pool(name="xtp", bufs=3))
            wkp = SA.enter_context(tc.tile_pool(name="wk", bufs=2))
            sqp = SA.enter_context(tc.tile_pool(name="sqp", bufs=1))
            brtok = SA.enter_context(tc.tile_pool(name="brtok", bufs=2))
            trps = SA.enter_context(tc.tile_pool(name="trps", bufs=2, space="PSUM"))
            haccp = SA.enter_context(tc.tile_pool(name="haccp", bufs=1, space="PSUM"))

            # broadcast cvec over partitions via PE outer product
            cps = haccp.tile([128, 512], F32, tag="cps")
            nc.tensor.matmul(cps[:, :64], ones1[:], cvec_sb[:])
            nc.vector.tensor_copy(C[:], cps[:, :64])

            wall_sb = wallp.tile([128, KT_IN * 32], F32R)
            nc.sync.dma_start(
                wall_sb[:], wall_d[:].rearrange("(kt p) m -> p kt m", p=128))

            # P1+P2: load x tiles; squares + per-tile rms; transpose token
            # pairs (256-wide) and accumulate h = wall.T @ xT (f32r, free 256).
            # As soon as a pair's h is done: hscal, ri, branch, brT quant for
            # its two tiles, so the fp8 MLP inputs are ready ASAP.
            h_ps = haccp.tile([128, 512], F32, tag="hps")
            ssq4 = scal.tile([128, TT * 4], F32)
            hT_sb = scal.tile([32, 512], F32, tag="hT")
            x_tok = []
            for t in range(TT):
                xt = xtokp.tile([128, NS * ED], F32, tag=f"xt{t}", name=f"xt{t}")
                nc.sync.dma_start(xt[:], x_d[t * 128:(t + 1) * 128])
                x_tok.append(xt)
                for j in range(NS):
                    sc = sqp.tile([128, ED], F32, tag="sq")
                    nc.scalar.activation(
                        sc[:], xt[:, j * ED:(j + 1) * ED], AF.Square,
                        accum_out=ssq4[:, t * 4 + j:t * 4 + j + 1])
                ssq1 = wkp.tile([128, 1], F32, tag="ssq1", name=f"ssq1_{t}")
                nc.vector.reduce_sum(ssq1[:], ssq4[:, t * 4:(t + 1) * 4],
                                     axis=mybir.AxisListType.X)
                vmean = wkp.tile([128, 1], F32, tag="vmean", name=f"vmean{t}")
                nc.scalar.activation(vmean[:], ssq1[:], AF.Copy,
                                     bias=EPS, scale=1.0 / IN_DIM)
                vinv = wkp.tile([128, 1], F32, tag="vinv", name=f"vinv{t}")
                nc.vector.reciprocal(vinv[:], vmean[:])
                nc.scalar.activation(rms[:, t:t + 1], vinv[:], AF.Sqrt)
                if t % 2 == 0:
                    continue
                pair = t // 2
                for fg in range(KT_IN // 4):
                    xt_ps = trps.tile([128, 1024], F32, tag="trp")
                    for tloc in range(2):
                        for kk in range(4):
                            ft = fg * 4 + kk
                            nc.tensor.transpose(
                                xt_ps[:, kk * 256 + tloc * 128:
                                      kk * 256 + (tloc + 1) * 128],
                                x_tok[pair * 2 + tloc][:, ft * 128:(ft + 1) * 128],
                                ident[:])
                    xt_sb = xtp.tile([128, 1024], F32R, tag="xt")
                    (nc.vector if fg % 2 == 0 else nc.gpsimd).tensor_copy(
                        xt_sb[:], xt_ps[:])
                    for kk in range(4):
                        ft = fg * 4 + kk
                        nc.tensor.matmul(
                            h_ps[:32, pair * 256:(pair + 1) * 256],
                            wall_sb[:, ft * 32:(ft + 1) * 32],
                            xt_sb[:, kk * 256:(kk + 1) * 256],
                            start=(ft == 0), stop=(ft == KT_IN - 1),
                            skip_group_check=True)

                # per-pair: h -> token-major, rms, ri gate, branch, brT quant
                nc.vector.tensor_copy(hT_sb[:, pair * 256:(pair + 1) * 256],
                                      h_ps[:32, pair * 256:(pair + 1) * 256])
                for tl in (t - 1, t):
                    hps2 = trps.tile([128, 512], F32, tag="btp")
                    nc.tensor.transpose(
                        hps2[:, :32], hT_sb[:, tl * 128:(tl + 1) * 128],
                        ident[:32, :32])
                    nc.vector.tensor_scalar_mul(
                        hscal[:, tl * 32:(tl + 1) * 32], hps2[:, :32],
                        rms[:, tl:tl + 1])
                    pr = soupp.tile([128, 4], F32, tag="w4a", name=f"w4a_{tl}")
                    nc.vector.scalar_tensor_tensor(
                        pr[:], hscal[:, tl * 32:tl * 32 + 4], C[:, 8:9],
                        C[:, 0:4], OP.mult, OP.add)
                    nc.scalar.activation(ri4[tl][:], pr[:], AF.Sigmoid)

                    br = brtok.tile([128, ED], F32, tag="br", name=f"br{tl}")
                    e0 = nc.vector if tl % 2 == 0 else nc.gpsimd
                    e0.tensor_scalar_mul(br[:], x_tok[tl][:, 0:ED],
                                         ri4[tl][:, 0:1])
                    for j in range(1, NS):
                        e0.scalar_tensor_tensor(
                            br[:], x_tok[tl][:, j * ED:(j + 1) * ED],
                            ri4[tl][:, j:j + 1], br[:], OP.mult, OP.add)
                    for kg in range(KT_ED // 4):
                        bt_ps = trps.tile([128, 512], F32, tag="btp")
                        for kk in range(4):
                            kt = kg * 4 + kk
                            nc.tensor.transpose(
                                bt_ps[:, kk * 128:(kk + 1) * 128],
                                br[:, kt * 128:(kt + 1) * 128], ident[:])
                        src = bt_ps[:].rearrange("p (four n) -> p four n",
                                                 four=4)
                        hi = brT_v[:, kg * 4:(kg + 1) * 4, 0,
                                   tl * 128:(tl + 1) * 128]
                        lo = brT_v[:, kg * 4:(kg + 1) * 4, 1,
                                   tl * 128:(tl + 1) * 128]
                        e1 = nc.gpsimd if kg % 2 == 0 else nc.vector
                        e1.tensor_copy(hi, src)
                        e1.tensor_sub(lo, src, hi)

        # ================= MLP + SOUP + MIX + TAIL =================
        with ExitStack() as SB:
            mixp = SB.enter_context(tc.tile_pool(name="mixp", bufs=1))
            yaccp = SB.enter_context(tc.tile_pool(name="yaccp", bufs=1))
            g1p = SB.enter_context(tc.tile_pool(name="g1p", bufs=1))
            w2sp = SB.enter_context(tc.tile_pool(name="w2s", bufs=2))
            mlpps = SB.enter_context(tc.tile_pool(name="mlpps", bufs=1, space="PSUM"))
            gfps = SB.enter_context(tc.tile_pool(name="gfps", bufs=2, space="PSUM"))
            SE = SB.enter_context(ExitStack())
            xhp = SE.enter_context(tc.tile_pool(name="xhp", bufs=1))
            w1sp = SE.enter_context(tc.tile_pool(name="w1s", bufs=2))

            mix = [mixp.tile([128, NS * ED], BF16, tag=f"mx{t}", name=f"mx{t}")
                   for t in range(TT)]
            yacc = yaccp.tile([128, KT_ED * TPC], F32)
            g1T = g1p.tile([128, KT_FDH * 2 * TPC], F8)
            g1T_v = g1T[:].rearrange("p (kt r n) -> p kt r n", kt=KT_FDH, r=2)

            def mlp1_half(half):
                for mgrp in range(half * 8, half * 8 + 8):  # groups of 512 dff
                    g_ps = [mlpps.tile([128, 512], F32, tag=f"mm{m}",
                                       name=f"gps{mgrp}_{m}") for m in range(4)]
                    w1t = w1sp.tile([128, KP_ED * 2 * 512], F8, tag="w1t")
                    nc.sync.dma_start(w1t[:], w1_d[mgrp])
                    w1v = w1t[:].rearrange("p (kp two m) -> p kp two m",
                                           kp=KP_ED, two=2)
                    for kp in range(KP_ED):
                        for m in range(4):
                            lhsT = w1v[:, kp, :, m * 128:(m + 1) * 128]
                            for role in range(2):
                                nc.tensor.matmul(
                                    g_ps[m][:], lhsT,
                                    brT_v[:, 2 * kp:2 * kp + 2, role, :],
                                    start=(kp == 0 and role == 0),
                                    stop=(kp == KP_ED - 1 and role == 1),
                                    perf_mode=DR)
                    for m in range(4):
                        mt = mgrp * 4 + m          # global dff m-tile
                        kth = mt - half * KT_FDH   # k-tile within half
                        gf = gfps.tile([128, 512], F32, tag="gf")
                        nc.scalar.activation(gf[:], g_ps[m][:],
                                             AF.Gelu_apprx_tanh,
                                             scale=1.0 / WSCALE)
                        hi = g1T_v[:, kth, 0, :]
                        lo = g1T_v[:, kth, 1, :]
                        if half == 0 and mgrp >= 4 and m % 2 == 0:
                            # Act helps with hi once the front drains
                            nc.scalar.activation(hi, gf[:], AF.Copy)
                            e2 = nc.gpsimd if m % 4 == 0 else nc.vector
                            e2.tensor_sub(lo, gf[:], hi)
                            continue
                        if half == 0:
                            e1 = nc.vector if m % 2 == 0 else nc.gpsimd
                            e2 = nc.gpsimd if m % 2 == 0 else nc.vector
                        else:
                            # DVE share queues behind the mix; route the
                            # first half to Pool, the rest to DVE
                            e1 = e2 = nc.gpsimd if mgrp < 12 else nc.vector
                        e1.tensor_copy(hi, gf[:])
                        e2.tensor_sub(lo, gf[:], hi)

            def mlp2_grp(half, mgrp2):
                y_ps = [mlpps.tile([128, 512], F32, tag=f"mm{m}",
                                   name=f"yps{half}_{mgrp2}_{m}")
                        for m in range(4)]
                w2v = []
                for kph in range(2):
                    w2t = w2sp.tile([128, 8 * 2 * 512], F8, tag="w2t")
                    nc.sync.dma_start(w2t[:], w2_d[half, mgrp2, kph])
                    w2v.append(w2t[:].rearrange(
                        "p (kp two m) -> p kp two m", kp=8, two=2))
                for kp in range(KP_FDH):
                    for m in range(4):
                        lhsT = w2v[kp // 8][:, kp % 8, :, m * 128:(m + 1) * 128]
                        for role in range(2):
                            nc.tensor.matmul(
                                y_ps[m][:], lhsT,
                                g1T_v[:, 2 * kp:2 * kp + 2, role, :],
                                start=(kp == 0 and role == 0),
                                stop=(kp == KP_FDH - 1 and role == 1),
                                perf_mode=DR)
                return y_ps

            # ---- MLP1 half 0 (g-quant split DVE/Pool) ----
            mlp1_half(0)

            # ---- rest of the scalar soup (overlaps MLP1 half 0) ----
            for t in range(TT):
                hs = hscal[:, t * 32:(t + 1) * 32]
                w = lambda cols, tg: soupp.tile([128, cols], F32, tag=tg,
                                                name=f"{tg}_{t}")

                pre_wo = w(4, "w4b")
                nc.vector.scalar_tensor_tensor(
                    pre_wo[:], hs[:, 4:8], C[:, 9:10], C[:, 4:8], OP.mult, OP.add)
                nc.scalar.activation(wo2[t][:], pre_wo[:], AF.Sigmoid)
                nc.scalar.mul(wo2[t][:], wo2[t][:], 2.0)

                pre_dt = w(2, "w2a")
                nc.vector.tensor_add(pre_dt[:], hs[:, 8:10], C[:, 10:12])
                sg = w(2, "w2b")
                nc.scalar.activation(sg[:], pre_dt[:], AF.Sigmoid)
                dt2 = w(2, "w2c")
                nc.scalar.activation(dt2[:], sg[:], AF.Copy,
                                     bias=DT_MIN, scale=DT_RANGE)

                pre_d = w(4, "w4c")
                nc.vector.tensor_add(pre_d[:], hs[:, 10:14], C[:, 12:16])
                esp = w(4, "w4f")
                nc.scalar.activation(esp[:], pre_d[:], AF.Exp)
                dsp = w(4, "w4d")
                nc.scalar.activation(dsp[:], esp[:], AF.Ln, bias=1.0)
                dscaled = w(4, "w4e")
                nc.vector.tensor_scalar_mul(dscaled[:], dsp[:], dt2[:, 1:2])
                ehD = w(4, f"ehD{t}")
                nc.scalar.activation(ehD[:], dscaled[:], AF.Exp, scale=-0.5)

                sdiff = w(6, "w6a")
                nc.vector.tensor_sub(sdiff[:], hs[:, 14:20], hs[:, 20:26])
                spre = w(6, "w6b")
                nc.vector.tensor_add(spre[:], sdiff[:], C[:, 16:22])
                s = w(6, f"s{t}")
                nc.vector.tensor_scalar_mul(s[:], spre[:], dt2[:, 0:1])

                sq = w(6, "w6c")
                nc.vector.tensor_mul(sq[:], s[:], s[:])
                p1 = w(1, "p1")
                nc.vector.reduce_sum(p1[:], sq[:], axis=mybir.AxisListType.X)
                prod3 = w(3, "w3a")
                nc.vector.tensor_mul(prod3[:], s[:, 0:3], s[:, 3:6])
                t1 = w(1, "t1")
                nc.vector.tensor_sub(t1[:], prod3[:, 0:1], prod3[:, 1:2])
                Pf = w(1, "Pf")
                nc.vector.tensor_add(Pf[:], t1[:], prod3[:, 2:3])
                q1 = w(1, "q1")
                nc.vector.tensor_mul(q1[:], Pf[:], Pf[:])
                Dm = w(1, "Dm")
                nc.vector.tensor_add(Dm[:], p1[:], q1[:])
                D1 = w(1, "D1")
                nc.scalar.activation(D1[:], Dm[:], AF.Copy, bias=1.0)
                r0 = w(1, "r0")
                nc.vector.reciprocal(r0[:], D1[:])
                t2 = w(1, "t2")
                nc.vector.tensor_mul(t2[:], D1[:], r0[:])
                t3 = w(1, "t3")
                nc.scalar.activation(t3[:], t2[:], AF.Copy, scale=-1.0, bias=2.0)
                invD = w(1, "invD")
                nc.vector.tensor_mul(invD[:], r0[:], t3[:])

                pr1 = w(2, "pr1")
                nc.vector.tensor_mul(pr1[:], s[:, 0:2], s[:, 4:6])
                pr2 = w(4, "pr2")
                nc.vector.tensor_mul(pr2[:], s[:, 0:4], s[:, 2:6])
                pr3 = w(5, "pr3")
                nc.vector.tensor_mul(pr3[:], s[:, 0:5], s[:, 1:6])
                pr4 = w(1, "pr4")
                nc.vector.tensor_mul(pr4[:], s[:, 0:1], s[:, 5:6])

                cE = w(6, "cE")
                g01 = w(1, "g01")
                nc.vector.tensor_add(g01[:], pr1[:, 1:2], pr2[:, 2:3])
                nc.scalar.activation(cE[:, 0:1], g01[:], AF.Copy, scale=-1.0)
                nc.vector.tensor_sub(cE[:, 1:2], pr4[:, 0:1], pr3[:, 2:3])
                nc.vector.tensor_add(cE[:, 2:3], pr1[:, 0:1], pr2[:, 1:2])
                g23 = w(1, "g23")
                nc.vector.tensor_add(g23[:], pr3[:, 1:2], pr3[:, 4:5])
                nc.scalar.activation(cE[:, 3:4], g23[:], AF.Copy, scale=-1.0)
                nc.vector.tensor_sub(cE[:, 4:5], pr2[:, 3:4], pr2[:, 0:1])
                g12 = w(1, "g12")
                nc.vector.tensor_add(g12[:], pr3[:, 0:1], pr3[:, 3:4])
                nc.scalar.activation(cE[:, 5:6], g12[:], AF.Copy, scale=-1.0)

                mdiag = w(4, "mdiag")
                nc.vector.reduce_sum(mdiag[:, 0:1], sq[:, 0:3],
                                     axis=mybir.AxisListType.X)
                m1a = w(1, "m1a")
                nc.vector.reduce_sum(m1a[:], sq[:, 4:6], axis=mybir.AxisListType.X)
                nc.vector.tensor_add(mdiag[:, 1:2], m1a[:], sq[:, 0:1])
                u1 = w(1, "u1")
                nc.vector.tensor_add(u1[:], sq[:, 1:2], sq[:, 3:4])
                nc.vector.tensor_add(mdiag[:, 2:3], u1[:], sq[:, 5:6])
                nc.vector.reduce_sum(mdiag[:, 3:4], sq[:, 2:5],
                                     axis=mybir.AxisListType.X)

                st6 = w(6, "st6")
                nc.vector.tensor_mul(st6[:, 0:3], s[:, 3:6], C[:, 22:25])
                nc.vector.tensor_mul(st6[:, 3:6], s[:, 0:3], C[:, 22:25])
                o6 = w(6, "o6")
                nc.vector.scalar_tensor_tensor(
                    o6[:], st6[:], Pf[:], s[:], OP.mult, OP.add)
                nplus = w(6, "npl")
                nc.vector.tensor_add(nplus[:], cE[:], o6[:])
                nminus = w(6, "nmi")
                nc.vector.tensor_sub(nminus[:], cE[:], o6[:])

                Ppair = w(6, "Ppair")
                for k, (i, j) in enumerate(PAIRS):
                    nc.vector.tensor_mul(
                        Ppair[:, k:k + 1], ehD[:, i:i + 1], ehD[:, j:j + 1])
                nc.vector.tensor_scalar_mul(Ppair[:], Ppair[:], invD[:])
                nc.scalar.mul(Ppair[:], Ppair[:], 2.0)
                nc.vector.tensor_mul(PhiP[t][:], Ppair[:], nplus[:])
                nc.vector.tensor_mul(PhiM[t][:], Ppair[:], nminus[:])

                base = w(1, "base")
                nc.vector.tensor_sub(base[:], p1[:], q1[:])
                base1 = w(1, "base1")
                nc.scalar.activation(base1[:], base[:], AF.Copy, bias=1.0)
                m2n = w(4, "m2n")
                nc.scalar.mul(m2n[:], mdiag[:], -2.0)
                numd = w(4, "numd")
                nc.vector.tensor_scalar_add(numd[:], m2n[:], base1[:])
                e2 = w(4, "e2")
                nc.vector.tensor_mul(e2[:], ehD[:], ehD[:])
                e2i = w(4, "e2i")
                nc.vector.tensor_scalar_mul(e2i[:], e2[:], invD[:])
                nc.vector.tensor_mul(PhiD[t][:], e2i[:], numd[:])

            # ---- MLP2 half 0 -> yacc ----
            for mgrp2 in range(4):
                y_ps = mlp2_grp(0, mgrp2)
                for m in range(4):
                    mt2 = mgrp2 * 4 + m
                    nc.scalar.activation(
                        yacc[:, mt2 * TPC:(mt2 + 1) * TPC], y_ps[m][:],
                        AF.Copy, scale=1.0 / WSCALE)

            # ---- mix on DVE (bf16, 2x rate), overlaps MLP ----
            for t in range(TT):
                xht = xhp.tile([128, NS * ED], BF16, tag="xh", name=f"xh{t}")
                nc.sync.dma_start(xht[:], xh_d[t * 128:(t + 1) * 128])
                for i in range(NS):
                    acc = mix[t][:, i * ED:(i + 1) * ED]
                    nc.vector.tensor_scalar_mul(
                        acc, xht[:, 0:ED], phi_ap(t, i, 0))
                    for j in range(1, NS):
                        nc.vector.scalar_tensor_tensor(
                            acc, xht[:, j * ED:(j + 1) * ED],
                            phi_ap(t, i, j), acc, OP.mult, OP.add)

            # ---- MLP1 half 1 (g-quant on Pool) ----
            mlp1_half(1)
            SE.close()

            # ---- MLP2 half 1 fused with the tail, per embed chunk ----
            with ExitStack() as SC:
                outp = SC.enter_context(tc.tile_pool(name="outp", bufs=2))
                tps = SC.enter_context(tc.tile_pool(name="tps", bufs=2,
                                                    space="PSUM"))
                for mgrp2 in range(4):
                    y_ps = mlp2_grp(1, mgrp2)
                    for m in range(4):
                        mt2 = mgrp2 * 4 + m
                        dst = yacc[:, mt2 * TPC:(mt2 + 1) * TPC]
                        nc.gpsimd.scalar_tensor_tensor(
                            dst, y_ps[m][:], 1.0 / WSCALE, dst, OP.mult, OP.add)
                    # tail for embed chunk mgrp2: transpose y, fuse wo*y + mix
                    for t in range(TT):
                        yt_ps = tps.tile([128, 512], F32, tag="tp",
                                         name=f"ytps{t}_{mgrp2}")
                        for kk in range(4):
                            m2 = mgrp2 * 4 + kk
                            nc.tensor.transpose(
                                yt_ps[:, kk * 128:(kk + 1) * 128],
                                yacc[:, m2 * TPC + t * 128:
                                     m2 * TPC + (t + 1) * 128], ident[:])
                        ot = outp.tile([128, NS * 512], F32, tag="ot",
                                       name=f"ot{t}_{mgrp2}")
                        for i in range(NS):
                            e1 = nc.vector if i % 2 == 0 else nc.gpsimd
                            e1.scalar_tensor_tensor(
                                ot[:, i * 512:(i + 1) * 512], yt_ps[:],
                                wo2[t][:, i:i + 1],
                                mix[t][:, i * ED + mgrp2 * 512:
                                       i * ED + (mgrp2 + 1) * 512],
                                OP.mult, OP.add)
                        nc.sync.dma_start(
                            out_d[t * 128:(t + 1) * 128, :,
                                  mgrp2 * 512:(mgrp2 + 1) * 512], ot[:])

    nc.compile()
    return nc


_NC_CACHE = None


def _get_nc():
    global _NC_CACHE
    if _NC_CACHE is None:
        _NC_CACHE = build_nc()
    return _NC_CACHE


def prep_inputs(inputs):
    """Host-side prep: pack weights/constants for the kernel."""
    f = lambda k: np.ascontiguousarray(np.asarray(inputs[k], np.float32))
    wall = np.zeros((IN_DIM, 32), np.float32)
    wall[:, 0:4] = f("W_ri").T
    wall[:, 4:8] = f("W_wo").T
    wall[:, 8] = f("W_dtc")[0]
    wall[:, 9] = f("W_dtd")[0]
    wall[:, 10:14] = f("W_diss").T
    wall[:, 14:20] = 0.5 * f("W_conv")[U_ROWS].T
    wall[:, 20:26] = 0.5 * f("W_conv")[L_ROWS].T
    A = f("conserv_A")
    cvec = np.zeros((1, 64), np.float32)
    cvec[0, 0:4] = f("read_in_p")[:, 0]
    cvec[0, 4:8] = f("write_out_p")[:, 0]
    cvec[0, 8] = f("alpha_read_in")[0]
    cvec[0, 9] = f("alpha_write_out")[0]
    cvec[0, 10] = f("log_dt_conserv")[0] + f("b_dtc")[0]
    cvec[0, 11] = f("log_dt_diss")[0] + f("b_dtd")[0]
    cvec[0, 12:16] = f("diss_diag")
    cvec[0, 16:22] = [0.5 * (A[i, j] - A[j, i]) for (i, j) in PAIRS]
    cvec[0, 22:25] = [1.0, -1.0, 1.0]

    # W1 [K=2048, M=8192]: k = kp*256 + ktl*128 + p; m = mgrp*512 + mm
    # -> [mgrp(16), p(128), kp(8), ktl(2), mm(512)], x256, fp8
    w1 = (f("W1") * WSCALE).reshape(KP_ED, 2, 128, 16, 512)
    w1 = np.ascontiguousarray(w1.transpose(3, 2, 0, 1, 4))
    w1 = w1.astype(ml_dtypes.float8_e4m3)
    # W2 [K=8192, M=2048]: k = half*4096 + kph*2048 + kp*256 + ktl*128 + p
    # m = mgrp2*512 + mm -> [half, mgrp2(4), kph(2), p, kp(8), ktl(2), mm(512)]
    w2 = (f("W2") * WSCALE).reshape(2, 2, 8, 2, 128, 4, 512)
    w2 = np.ascontiguousarray(w2.transpose(0, 5, 1, 4, 2, 3, 6))
    w2 = w2.astype(ml_dtypes.float8_e4m3)
    x = f("x")
    xh = np.ascontiguousarray(x.astype(ml_dtypes.bfloat16))
    return {
        "wall": np.ascontiguousarray(wall),
        "w1": w1,
        "w2": w2,
        "cvec": cvec,
        "x": x,
        "xh": xh,
    }


def kernel(**inputs) -> np.ndarray:
    prep = prep_inputs(inputs)
    x = prep["x"]
    xh = prep["xh"]
    nc = _get_nc()
    in_maps = []
    for c in range(NCORES):
        in_maps.append({
            "x": np.ascontiguousarray(x[c * TPC:(c + 1) * TPC]),
            "xh": np.ascontiguousarray(xh[c * TPC:(c + 1) * TPC]),
            "wall": prep["wall"],
            "w1": prep["w1"],
            "w2": prep["w2"],
            "cvec": prep["cvec"],
        })
    res = run_bass_kernel_spmd(nc, in_maps, list(range(NCORES)))
    out = np.concatenate([res.results[c]["out"] for c in range(NCORES)], axis=0)
    return out.astype(np.float32)


# revision 16
# speedup vs baseline: 1.0028x; 1.0028x over previous
"""Trainium2 Bass kernel for nn_ContinuousGenHyperConnectionsStrang.

Contract: kernel(**inputs) takes FULL unsharded inputs (as in
reference.setup_inputs()) and returns the FULL [4096, 4, 2048] f32 output.

Strategy (8 cores, data-parallel over tokens, 512 tokens/core):
  - Front: x loaded token-major (4 tiles resident); RMS stats + h_all =
    xn @ W_all.T in f32r (PE transposes, 256-wide token pairs). Only the
    read-in gates ri are computed before the MLP; the rest of the per-token
    generator scalars (4x4 Cayley closed form) runs overlapped with MLP1.
  - branch = sum_n ri_n x_n quantized to fp8-e4m3 hi+lo pairs (quant error
    compensated); W1/W2 pre-scaled x256 and cast to fp8 on the host.
  - MLP entirely fp8 DoubleRow (2 k-planes per matmul instruction): MLP1
    computes branch-hi and branch-lo products; g1 = gelu re-quantized hi+lo;
    MLP2 likewise. Residual error is weight quantization only (~1.6e-2 rel).
  - Stream mix out_i = sum_j Phi_ij x_j computed in bf16 (2x DVE rate) from
    a host-cast bf16 copy of x, overlapped with MLP; the tail is fused into
    MLP2 half 1 per 512-wide embed chunk (transpose y from PSUM, STT with
    wo_i straight out of PSUM, chunked output DMAs).
"""
import numpy as np
import ml_dtypes

import concourse.bass as bass
import concourse.bacc as bacc
import concourse.mybir as mybir
import concourse.tile as tile
from concourse.bass_utils import run_bass_kernel_spmd
from concourse.masks import make_identity
from contextlib import ExitStack

F32 = mybir.dt.float32
F32R = mybir.dt.float32r
BF16 = mybir.dt.bfloat16
F8 = mybir.dt.float8e4
AF = mybir.ActivationFunctionType
OP = mybir.AluOpType
DR = mybir.MatmulPerfMode.DoubleRow

NCORES = 8
B_FULL = 4096
TPC = B_FULL // NCORES          # 512 tokens per core
TT = TPC // 128                 # 4 token tiles
NS = 4                          # streams (N)
ED = 2048                       # EMBED / BLOCK
IN_DIM = NS * ED                # 8192
FD = 8192                       # DFF
KT_IN = IN_DIM // 128           # 64 k-tiles over input dim
KT_ED = ED // 128               # 16 k-tiles over embed
KP_ED = KT_ED // 2              # 8 k-pairs over embed
KT_FDH = 32                     # dff k-tiles per half
KP_FDH = KT_FDH // 2            # 16 k-pairs per dff half
WSCALE = 256.0
DT_MIN, DT_MAX = 1e-3, 1.0
DT_RANGE = DT_MAX - DT_MIN
EPS = 1.1920929e-7

PAIRS = [(0, 1), (0, 2), (0, 3), (2, 3), (1, 3), (1, 2)]
PIDX = {p: k for k, p in enumerate(PAIRS)}
U_ROWS = [4 * i + j for (i, j) in PAIRS]
L_ROWS = [4 * j + i for (i, j) in PAIRS]


def build_nc():
    nc = bacc.Bacc()
    x_d = nc.declare_dram_parameter("x", [TPC, NS, ED], F32, isOutput=False)
    xh_d = nc.declare_dram_parameter("xh", [TPC, NS, ED], BF16, isOutput=False)
    wall_d = nc.declare_dram_parameter("wall", [IN_DIM, 32], F32R, isOutput=False)
    # w1: [mgrp(16), p(128), kp(8) x ktl(2) x m(512)] - one DMA per mgrp
    w1_d = nc.declare_dram_parameter("w1", [16, 128, KP_ED * 2 * 512], F8,
                                     isOutput=False)
    # w2: [half(2), mgrp2(4), kph(2), p(128), kp(8) x ktl(2) x m(512)]
    w2_d = nc.declare_dram_parameter("w2", [2, 4, 2, 128, 8 * 2 * 512], F8,
                                     isOutput=False)
    cvec_d = nc.declare_dram_parameter("cvec", [1, 64], F32, isOutput=False)
    out_d = nc.declare_dram_parameter("out", [TPC, NS, ED], F32, isOutput=True)

    with tile.TileContext(nc) as tc, ExitStack() as S0:
        const = S0.enter_context(tc.tile_pool(name="const", bufs=1))
        scal = S0.enter_context(tc.tile_pool(name="scal", bufs=1))
        soupp = S0.enter_context(tc.tile_pool(name="soupp", bufs=4))
        brp = S0.enter_context(tc.tile_pool(name="brp", bufs=1))

        ident = const.tile([128, 128], F32)
        make_identity(nc, ident[:])
        ones1 = const.tile([1, 128], F32)
        nc.gpsimd.memset(ones1[:], 1.0)
        cvec_sb = const.tile([1, 64], F32)
        nc.sync.dma_start(cvec_sb[:], cvec_d[:])

        # persistent per-token scalar outputs
        C = const.tile([128, 64], F32)
        rms = scal.tile([128, TT], F32)
        hscal = scal.tile([128, 32 * TT], F32)
        ri4 = [scal.tile([128, 4], F32, tag=f"ri{t}", name=f"ri{t}") for t in range(TT)]
        wo2 = [scal.tile([128, 4], F32, tag=f"wo{t}", name=f"wo{t}") for t in range(TT)]
        PhiP = [scal.tile([128, 6], F32, tag=f"pp{t}", name=f"pp{t}") for t in range(TT)]
        PhiM = [scal.tile([128, 6], F32, tag=f"pm{t}", name=f"pm{t}") for t in range(TT)]
        PhiD = [scal.tile([128, 4], F32, tag=f"pd{t}", name=f"pd{t}") for t in range(TT)]

        def phi_ap(t, i, j):
            if i == j:
                return PhiD[t][:, i:i + 1]
            if (i, j) in PIDX:
                return PhiP[t][:, PIDX[(i, j)]:PIDX[(i, j)] + 1]
            return PhiM[t][:, PIDX[(j, i)]:PIDX[(j, i)] + 1]

        # branchT fp8 hi+lo: [128(k), kt(16) x role(2) x tok(512)] = 16KB
        brT = brp.tile([128, KT_ED * 2 * TPC], F8)
        brT_v = brT[:].rearrange("p (kt r n) -> p kt r n", kt=KT_ED, r=2)

        # ================= FRONT =================
        with ExitStack() as SA:
            xtokp = SA.enter_context(tc.tile_pool(name="xtok", bufs=1))
            wallp = SA.enter_context(tc.tile_pool(name="wallp", bufs=1))
            xtp = SA.enter_context(tc.tile_pool(name="xtp", bufs=3))
            wkp = SA.enter_context(tc.tile_pool(name="wk", bufs=2))
            sqp = SA.enter_context(tc.tile_pool(name="sqp", bufs=1))
            brtok = SA.enter_context(tc.tile_pool(name="brtok", bufs=2))
            trps = SA.enter_context(tc.tile_pool(name="trps", bufs=2, space="PSUM"))
            haccp = SA.enter_context(tc.tile_pool(name="haccp", bufs=1, space="PSUM"))

            # broadcast cvec over partitions via PE outer product
            cps = haccp.tile([128, 512], F32, tag="cps")
            nc.tensor.matmul(cps[:, :64], ones1[:], cvec_sb[:])
            nc.vector.tensor_copy(C[:], cps[:, :64])

            wall_sb = wallp.tile([128, KT_IN * 32], F32R)
            nc.sync.dma_start(
                wall_sb[:], wall_d[:].rearrange("(kt p) m -> p kt m", p=128))

            # P1+P2: load x tiles; squares + per-tile rms; transpose token
            # pairs (256-wide) and accumulate h = wall.T @ xT (f32r, free 256).
            # As soon as a pair's h is done: hscal, ri, branch, brT quant for
            # its two tiles, so the fp8 MLP inputs are ready ASAP.
            h_ps = haccp.tile([128, 512], F32, tag="hps")
            ssq4 = scal.tile([128, TT * 4], F32)
            hT_sb = scal.tile([32, 512], F32, tag="hT")
            x_tok = []
            for t in range(TT):
                xt = xtokp.tile([128, NS * ED], F32, tag=f"xt{t}", name=f"xt{t}")
                nc.sync.dma_start(xt[:], x_d[t * 128:(t + 1) * 128])
                x_tok.append(xt)
                for j in range(NS):
                    sc = sqp.tile([128, ED], F32, tag="sq")
                    nc.scalar.activation(
                        sc[:], xt[:, j * ED:(j + 1) * ED], AF.Square,
                        accum_out=ssq4[:, t * 4 + j:t * 4 + j + 1])
                ssq1 = wkp.tile([128, 1], F32, tag="ssq1", name=f"ssq1_{t}")
                nc.vector.reduce_sum(ssq1[:], ssq4[:, t * 4:(t + 1) * 4],
                                     axis=mybir.AxisListType.X)
                vmean = wkp.tile([128, 1], F32, tag="vmean", name=f"vmean{t}")
                nc.scalar.activation(vmean[:], ssq1[:], AF.Copy,
                                     bias=EPS, scale=1.0 / IN_DIM)
                vinv = wkp.tile([128, 1], F32, tag="vinv", name=f"vinv{t}")
                nc.vector.reciprocal(vinv[:], vmean[:])
                nc.scalar.activation(rms[:, t:t + 1], vinv[:], AF.Sqrt)
                if t % 2 == 0:
                    continue
                pair = t // 2
                for fg in range(KT_IN // 4):
                    xt_ps = trps.tile([128, 1024], F32, tag="trp")
                    for tloc in range(2):
                        for kk in range(4):
                            ft = fg * 4 + kk
                            nc.tensor.transpose(
                                xt_ps[:, kk * 256 + tloc * 128:
                                      kk * 256 + (tloc + 1) * 128],
                                x_tok[pair * 2 + tloc][:, ft * 128:(ft + 1) * 128],
                                ident[:])
                    xt_sb = xtp.tile([128, 1024], F32R, tag="xt")
                    (nc.vector if fg % 2 == 0 else nc.gpsimd).tensor_copy(
                        xt_sb[:], xt_ps[:])
                    for kk in range(4):
                        ft = fg * 4 + kk
                        nc.tensor.matmul(
                            h_ps[:32, pair * 256:(pair + 1) * 256],
                            wall_sb[:, ft * 32:(ft + 1) * 32],
                            xt_sb[:, kk * 256:(kk + 1) * 256],
                            start=(ft == 0), stop=(ft == KT_IN - 1),
                            skip_group_check=True)

                # per-pair: h -> token-major, rms, ri gate, branch, brT quant
                nc.vector.tensor_copy(hT_sb[:, pair * 256:(pair + 1) * 256],
                                      h_ps[:32, pair * 256:(pair + 1) * 256])
                for tl in (t - 1, t):
                    hps2 = trps.tile([128, 512], F32, tag="btp")
                    nc.tensor.transpose(
                        hps2[:, :32], hT_sb[:, tl * 128:(tl + 1) * 128],
                        ident[:32, :32])
                    nc.vector.tensor_scalar_mul(
                        hscal[:, tl * 32:(tl + 1) * 32], hps2[:, :32],
                        rms[:, tl:tl + 1])
                    pr = soupp.tile([128, 4], F32, tag="w4a", name=f"w4a_{tl}")
                    nc.vector.scalar_tensor_tensor(
                        pr[:], hscal[:, tl * 32:tl * 32 + 4], C[:, 8:9],
                        C[:, 0:4], OP.mult, OP.add)
                    nc.scalar.activation(ri4[tl][:], pr[:], AF.Sigmoid)

                    br = brtok.tile([128, ED], F32, tag="br", name=f"br{tl}")
                    e0 = nc.vector if tl % 2 == 0 else nc.gpsimd
                    e0.tensor_scalar_mul(br[:], x_tok[tl][:, 0:ED],
                                         ri4[tl][:, 0:1])
                    for j in range(1, NS):
                        e0.scalar_tensor_tensor(
                            br[:], x_tok[tl][:, j * ED:(j + 1) * ED],
                            ri4[tl][:, j:j + 1], br[:], OP.mult, OP.add)
                    for kg in range(KT_ED // 4):
                        bt_ps = trps.tile([128, 512], F32, tag="btp")
                        for kk in range(4):
                            kt = kg * 4 + kk
                            nc.tensor.transpose(
                                bt_ps[:, kk * 128:(kk + 1) * 128],
                                br[:, kt * 128:(kt + 1) * 128], ident[:])
                        src = bt_ps[:].rearrange("p (four n) -> p four n",
                                                 four=4)
                        hi = brT_v[:, kg * 4:(kg + 1) * 4, 0,
                                   tl * 128:(tl + 1) * 128]
                        lo = brT_v[:, kg * 4:(kg + 1) * 4, 1,
                                   tl * 128:(tl + 1) * 128]
                        e1 = nc.gpsimd if kg % 2 == 0 else nc.vector
                        e1.tensor_copy(hi, src)
                        e1.tensor_sub(lo, src, hi)

        # ================= MLP + SOUP + MIX + TAIL =================
        with ExitStack() as SB:
            mixp = SB.enter_context(tc.tile_pool(name="mixp", bufs=1))
            yaccp = SB.enter_context(tc.tile_pool(name="yaccp", bufs=1))
            g1p = SB.enter_context(tc.tile_pool(name="g1p", bufs=1))
            w2sp = SB.enter_context(tc.tile_pool(name="w2s", bufs=2))
            mlpps = SB.enter_context(tc.tile_pool(name="mlpps", bufs=1, space="PSUM"))
            gfps = SB.enter_context(tc.tile_pool(name="gfps", bufs=2, space="PSUM"))
            SE = SB.enter_context(ExitStack())
            xhp = SE.enter_context(tc.tile_pool(name="xhp", bufs=1))
            w1sp = SE.enter_context(tc.tile_pool(name="w1s", bufs=2))

            mix = [mixp.tile([128, NS * ED], BF16, tag=f"mx{t}", name=f"mx{t}")
                   for t in range(TT)]
            yacc = yaccp.tile([128, KT_ED * TPC], F32)
            g1T = g1p.tile([128, KT_FDH * 2 * TPC], F8)
            g1T_v = g1T[:].rearrange("p (kt r n) -> p kt r n", kt=KT_FDH, r=2)

            def mlp1_half(half):
                for mgrp in range(half * 8, half * 8 + 8):  # groups of 512 dff
                    g_ps = [mlpps.tile([128, 512], F32, tag=f"mm{m}",
                                       name=f"gps{mgrp}_{m}") for m in range(4)]
                    w1t = w1sp.tile([128, KP_ED * 2 * 512], F8, tag="w1t")
                    nc.sync.dma_start(w1t[:], w1_d[mgrp])
                    w1v = w1t[:].rearrange("p (kp two m) -> p kp two m",
                                           kp=KP_ED, two=2)
                    for kp in range(KP_ED):
                        for m in range(4):
                            lhsT = w1v[:, kp, :, m * 128:(m + 1) * 128]
                            for role in range(2):
                                nc.tensor.matmul(
                                    g_ps[m][:], lhsT,
                                    brT_v[:, 2 * kp:2 * kp + 2, role, :],
                                    start=(kp == 0 and role == 0),
                                    stop=(kp == KP_ED - 1 and role == 1),
                                    perf_mode=DR)
                    for m in range(4):
                        mt = mgrp * 4 + m          # global dff m-tile
                        kth = mt - half * KT_FDH   # k-tile within half
                        gf = gfps.tile([128, 512], F32, tag="gf")
                        nc.scalar.activation(gf[:], g_ps[m][:],
                                             AF.Gelu_apprx_tanh,
                                             scale=1.0 / WSCALE)
                        hi = g1T_v[:, kth, 0, :]
                        lo = g1T_v[:, kth, 1, :]
                        # hi on Act (it has slack next to the gelus); lo on
                        # DVE/Pool. DVE's half-1 share queues behind the mix,
                        # so route early half-1 groups to Pool.
                        nc.scalar.activation(hi, gf[:], AF.Copy)
                        if half == 0:
                            e2 = nc.gpsimd if m % 2 == 0 else nc.vector
                        else:
                            e2 = nc.gpsimd if mgrp < 12 else nc.vector
                        e2.tensor_sub(lo, gf[:], hi)

            def mlp2_grp(half, mgrp2):
                y_ps = [mlpps.tile([128, 512], F32, tag=f"mm{m}",
                                   name=f"yps{half}_{mgrp2}_{m}")
                        for m in range(4)]
                w2v = []
                for kph in range(2):
                    w2t = w2sp.tile([128, 8 * 2 * 512], F8, tag="w2t")
                    nc.sync.dma_start(w2t[:], w2_d[half, mgrp2, kph])
                    w2v.append(w2t[:].rearrange(
                        "p (kp two m) -> p kp two m", kp=8, two=2))
                for kp in range(KP_FDH):
                    for m in range(4):
                        lhsT = w2v[kp // 8][:, kp % 8, :, m * 128:(m + 1) * 128]
                        for role in range(2):
                            nc.tensor.matmul(
                                y_ps[m][:], lhsT,
                                g1T_v[:, 2 * kp:2 * kp + 2, role, :],
                                start=(kp == 0 and role == 0),
                                stop=(kp == KP_FDH - 1 and role == 1),
                                perf_mode=DR)
                return y_ps

            # ---- MLP1 half 0 (g-quant split DVE/Pool) ----
            mlp1_half(0)

            # ---- rest of the scalar soup (overlaps MLP1 half 0) ----
            for t in range(TT):
                hs = hscal[:, t * 32:(t + 1) * 32]
                w = lambda cols, tg: soupp.tile([128, cols], F32, tag=tg,
                                                name=f"{tg}_{t}")

                pre_wo = w(4, "w4b")
                nc.vector.scalar_tensor_tensor(
                    pre_wo[:], hs[:, 4:8], C[:, 9:10], C[:, 4:8], OP.mult, OP.add)
                nc.scalar.activation(wo2[t][:], pre_wo[:], AF.Sigmoid)
                nc.scalar.mul(wo2[t][:], wo2[t][:], 2.0)

                pre_dt = w(2, "w2a")
                nc.vector.tensor_add(pre_dt[:], hs[:, 8:10], C[:, 10:12])
                sg = w(2, "w2b")
                nc.scalar.activation(sg[:], pre_dt[:], AF.Sigmoid)
                dt2 = w(2, "w2c")
                nc.scalar.activation(dt2[:], sg[:], AF.Copy,
                                     bias=DT_MIN, scale=DT_RANGE)

                pre_d = w(4, "w4c")
                nc.vector.tensor_add(pre_d[:], hs[:, 10:14], C[:, 12:16])
                esp = w(4, "w4f")
                nc.scalar.activation(esp[:], pre_d[:], AF.Exp)
                dsp = w(4, "w4d")
                nc.scalar.activation(dsp[:], esp[:], AF.Ln, bias=1.0)
                dscaled = w(4, "w4e")
                nc.vector.tensor_scalar_mul(dscaled[:], dsp[:], dt2[:, 1:2])
                ehD = w(4, f"ehD{t}")
                nc.scalar.activation(ehD[:], dscaled[:], AF.Exp, scale=-0.5)

                sdiff = w(6, "w6a")
                nc.vector.tensor_sub(sdiff[:], hs[:, 14:20], hs[:, 20:26])
                spre = w(6, "w6b")
                nc.vector.tensor_add(spre[:], sdiff[:], C[:, 16:22])
                s = w(6, f"s{t}")
                nc.vector.tensor_scalar_mul(s[:], spre[:], dt2[:, 0:1])

                sq = w(6, "w6c")
                nc.vector.tensor_mul(sq[:], s[:], s[:])
                p1 = w(1, "p1")
                nc.vector.reduce_sum(p1[:], sq[:], axis=mybir.AxisListType.X)
                prod3 = w(3, "w3a")
                nc.vector.tensor_mul(prod3[:], s[:, 0:3], s[:, 3:6])
                t1 = w(1, "t1")
                nc.vector.tensor_sub(t1[:], prod3[:, 0:1], prod3[:, 1:2])
                Pf = w(1, "Pf")
                nc.vector.tensor_add(Pf[:], t1[:], prod3[:, 2:3])
                q1 = w(1, "q1")
                nc.vector.tensor_mul(q1[:], Pf[:], Pf[:])
                Dm = w(1, "Dm")
                nc.vector.tensor_add(Dm[:], p1[:], q1[:])
                D1 = w(1, "D1")
                nc.scalar.activation(D1[:], Dm[:], AF.Copy, bias=1.0)
                r0 = w(1, "r0")
                nc.vector.reciprocal(r0[:], D1[:])
                t2 = w(1, "t2")
                nc.vector.tensor_mul(t2[:], D1[:], r0[:])
                t3 = w(1, "t3")
                nc.scalar.activation(t3[:], t2[:], AF.Copy, scale=-1.0, bias=2.0)
                invD = w(1, "invD")
                nc.vector.tensor_mul(invD[:], r0[:], t3[:])

                pr1 = w(2, "pr1")
                nc.vector.tensor_mul(pr1[:], s[:, 0:2], s[:, 4:6])
                pr2 = w(4, "pr2")
                nc.vector.tensor_mul(pr2[:], s[:, 0:4], s[:, 2:6])
                pr3 = w(5, "pr3")
                nc.vector.tensor_mul(pr3[:], s[:, 0:5], s[:, 1:6])
                pr4 = w(1, "pr4")
                nc.vector.tensor_mul(pr4[:], s[:, 0:1], s[:, 5:6])

                cE = w(6, "cE")
                g01 = w(1, "g01")
                nc.vector.tensor_add(g01[:], pr1[:, 1:2], pr2[:, 2:3])
                nc.scalar.activation(cE[:, 0:1], g01[:], AF.Copy, scale=-1.0)
                nc.vector.tensor_sub(cE[:, 1:2], pr4[:, 0:1], pr3[:, 2:3])
                nc.vector.tensor_add(cE[:, 2:3], pr1[:, 0:1], pr2[:, 1:2])
                g23 = w(1, "g23")
                nc.vector.tensor_add(g23[:], pr3[:, 1:2], pr3[:, 4:5])
                nc.scalar.activation(cE[:, 3:4], g23[:], AF.Copy, scale=-1.0)
                nc.vector.tensor_sub(cE[:, 4:5], pr2[:, 3:4], pr2[:, 0:1])
                g12 = w(1, "g12")
                nc.vector.tensor_add(g12[:], pr3[:, 0:1], pr3[:, 3:4])
                nc.scalar.activation(cE[:, 5:6], g12[:], AF.Copy, scale=-1.0)

                mdiag = w(4, "mdiag")
                nc.vector.reduce_sum(mdiag[:, 0:1], sq[:, 0:3],
                                     axis=mybir.AxisListType.X)
                m1a = w(1, "m1a")
                nc.vector.reduce_sum(m1a[:], sq[:, 4:6], axis=mybir.AxisListType.X)
                nc.vector.tensor_add(mdiag[:, 1:2], m1a[:], sq[:, 0:1])
                u1 = w(1, "u1")
                nc.vector.tensor_add(u1[:], sq[:, 1:2], sq[:, 3:4])
                nc.vector.tensor_add(mdiag[:, 2:3], u1[:], sq[:, 5:6])
                nc.vector.reduce_sum(mdiag[:, 3:4], sq[:, 2:5],
                                     axis=mybir.AxisListType.X)

                st6 = w(6, "st6")
                nc.vector.tensor_mul(st6[:, 0:3], s[:, 3:6], C[:, 22:25])
                nc.vector.tensor_mul(st6[:, 3:6], s[:, 0:3], C[:, 22:25])
                o6 = w(6, "o6")
                nc.vector.scalar_tensor_tensor(
                    o6[:], st6[:], Pf[:], s[:], OP.mult, OP.add)
                nplus = w(6, "npl")
                nc.vector.tensor_add(nplus[:], cE[:], o6[:])
                nminus = w(6, "nmi")
                nc.vector.tensor_sub(nminus[:], cE[:], o6[:])

                Ppair = w(6, "Ppair")
                for k, (i, j) in enumerate(PAIRS):
                    nc.vector.tensor_mul(
                        Ppair[:, k:k + 1], ehD[:, i:i + 1], ehD[:, j:j + 1])
                nc.vector.tensor_scalar_mul(Ppair[:], Ppair[:], invD[:])
                nc.scalar.mul(Ppair[:], Ppair[:], 2.0)
                nc.vector.tensor_mul(PhiP[t][:], Ppair[:], nplus[:])
                nc.vector.tensor_mul(PhiM[t][:], Ppair[:], nminus[:])

                base = w(1, "base")
                nc.vector.tensor_sub(base[:], p1[:], q1[:])
                base1 = w(1, "base1")
                nc.scalar.activation(base1[:], base[:], AF.Copy, bias=1.0)
                m2n = w(4, "m2n")
                nc.scalar.mul(m2n[:], mdiag[:], -2.0)
                numd = w(4, "numd")
                nc.vector.tensor_scalar_add(numd[:], m2n[:], base1[:])
                e2 = w(4, "e2")
                nc.vector.tensor_mul(e2[:], ehD[:], ehD[:])
                e2i = w(4, "e2i")
                nc.vector.tensor_scalar_mul(e2i[:], e2[:], invD[:])
                nc.vector.tensor_mul(PhiD[t][:], e2i[:], numd[:])

            # ---- MLP2 half 0 -> yacc ----
            for mgrp2 in range(4):
                y_ps = mlp2_grp(0, mgrp2)
                for m in range(4):
                    mt2 = mgrp2 * 4 + m
                    nc.scalar.activation(
                        yacc[:, mt2 * TPC:(mt2 + 1) * TPC], y_ps[m][:],
                        AF.Copy, scale=1.0 / WSCALE)

            # ---- mix on DVE (bf16, 2x rate), overlaps MLP ----
            for t in range(TT):
                xht = xhp.tile([128, NS * ED], BF16, tag="xh", name=f"xh{t}")
                nc.sync.dma_start(xht[:], xh_d[t * 128:(t + 1) * 128])
                for i in range(NS):
                    acc = mix[t][:, i * ED:(i + 1) * ED]
                    nc.vector.tensor_scalar_mul(
                        acc, xht[:, 0:ED], phi_ap(t, i, 0))
                    for j in range(1, NS):
                        nc.vector.scalar_tensor_tensor(
                            acc, xht[:, j * ED:(j + 1) * ED],
                            phi_ap(t, i, j), acc, OP.mult, OP.add)

            # ---- MLP1 half 1 (g-quant on Pool) ----
            mlp1_half(1)
            SE.close()

            # ---- MLP2 half 1 fused with the tail, per embed chunk ----
            with ExitStack() as SC:
                outp = SC.enter_context(tc.tile_pool(name="outp", bufs=2))
                tps = SC.enter_context(tc.tile_pool(name="tps", bufs=2,
                                                    space="PSUM"))
                for mgrp2 in range(4):
                    y_ps = mlp2_grp(1, mgrp2)
                    for m in range(4):
                        mt2 = mgrp2 * 4 + m
                        dst = yacc[:, mt2 * TPC:(mt2 + 1) * TPC]
                        nc.gpsimd.scalar_tensor_tensor(
                            dst, y_ps[m][:], 1.0 / WSCALE, dst, OP.mult, OP.add)
                    # tail for embed chunk mgrp2: transpose y, fuse wo*y + mix
                    for t in range(TT):
                        yt_ps = tps.tile([128, 512], F32, tag="tp",
                                         name=f"ytps{t}_{mgrp2}")
                        for kk in range(4):
                            m2 = mgrp2 * 4 + kk
                            nc.tensor.transpose(
                                yt_ps[:, kk * 128:(kk + 1) * 128],
                                yacc[:, m2 * TPC + t * 128:
                                     m2 * TPC + (t + 1) * 128], ident[:])
                        ot = outp.tile([128, NS * 512], F32, tag="ot",
                                       name=f"ot{t}_{mgrp2}")
                        for i in range(NS):
                            e1 = nc.vector if i % 2 == 0 else nc.gpsimd
                            e1.scalar_tensor_tensor(
                                ot[:, i * 512:(i + 1) * 512], yt_ps[:],
                                wo2[t][:, i:i + 1],
                                mix[t][:, i * ED + mgrp2 * 512:
                                       i * ED + (mgrp2 + 1) * 512],
                                OP.mult, OP.add)
                        nc.sync.dma_start(
                            out_d[t * 128:(t + 1) * 128, :,
                                  mgrp2 * 512:(mgrp2 + 1) * 512], ot[:])

    nc.compile()
    return nc


_NC_CACHE = None


def _get_nc():
    global _NC_CACHE
    if _NC_CACHE is None:
        _NC_CACHE = build_nc()
    return _NC_CACHE


def prep_inputs(inputs):
    """Host-side prep: pack weights/constants for the kernel."""
    f = lambda k: np.ascontiguousarray(np.asarray(inputs[k], np.float32))
    wall = np.zeros((IN_DIM, 32), np.float32)
    wall[:, 0:4] = f("W_ri").T
    wall[:, 4:8] = f("W_wo").T
    wall[:, 8] = f("W_dtc")[0]
    wall[:, 9] = f("W_dtd")[0]
    wall[:, 10:14] = f("W_diss").T
    wall[:, 14:20] = 0.5 * f("W_conv")[U_ROWS].T
    wall[:, 20:26] = 0.5 * f("W_conv")[L_ROWS].T
    A = f("conserv_A")
    cvec = np.zeros((1, 64), np.float32)
    cvec[0, 0:4] = f("read_in_p")[:, 0]
    cvec[0, 4:8] = f("write_out_p")[:, 0]
    cvec[0, 8] = f("alpha_read_in")[0]
    cvec[0, 9] = f("alpha_write_out")[0]
    cvec[0, 10] = f("log_dt_conserv")[0] + f("b_dtc")[0]
    cvec[0, 11] = f("log_dt_diss")[0] + f("b_dtd")[0]
    cvec[0, 12:16] = f("diss_diag")
    cvec[0, 16:22] = [0.5 * (A[i, j] - A[j, i]) for (i, j) in PAIRS]
    cvec[0, 22:25] = [1.0, -1.0, 1.0]

    # W1 [K=2048, M=8192]: k = kp*256 + ktl*128 + p; m = mgrp*512 + mm
    # -> [mgrp(16), p(128), kp(8), ktl(2), mm(512)], x256, fp8
    w1 = (f("W1") * WSCALE).reshape(KP_ED, 2, 128, 16, 512)
    w1 = np.ascontiguousarray(w1.transpose(3, 2, 0, 1, 4))
    w1 = w1.astype(ml_dtypes.float8_e4m3)
    # W2 [K=8192, M=2048]: k = half*4096 + kph*2048 + kp*256 + ktl*128 + p
    # m = mgrp2*512 + mm -> [half, mgrp2(4), kph(2), p, kp(8), ktl(2), mm(512)]
    w2 = (f("W2") * WSCALE).reshape(2, 2, 8, 2, 128, 4, 512)
    w2 = np.ascontiguousarray(w2.transpose(0, 5, 1, 4, 2, 3, 6))
    w2 = w2.astype(ml_dtypes.float8_e4m3)
    x = f("x")
    xh = np.ascontiguousarray(x.astype(ml_dtypes.bfloat16))
    return {
        "wall": np.ascontiguousarray(wall),
        "w1": w1,
        "w2": w2,
        "cvec": cvec,
        "x": x,
        "xh": xh,
    }


def kernel(**inputs) -> np.ndarray:
    prep = prep_inputs(inputs)
    x = prep["x"]
    xh = prep["xh"]
    nc = _get_nc()
    in_maps = []
    for c in range(NCORES):
        in_maps.append({
            "x": np.ascontiguousarray(x[c * TPC:(c + 1) * TPC]),
            "xh": np.ascontiguousarray(xh[c * TPC:(c + 1) * TPC]),
            "wall": prep["wall"],
            "w1": prep["w1"],
            "w2": prep["w2"],
            "cvec": prep["cvec"],
        })
    res = run_bass_kernel_spmd(nc, in_maps, list(range(NCORES)))
    out = np.concatenate([res.results[c]["out"] for c in range(NCORES)], axis=0)
    return out.astype(np.float32)


# revision 21
# speedup vs baseline: 1.0075x; 1.0046x over previous
"""Trainium2 Bass kernel for nn_ContinuousGenHyperConnectionsStrang.

Contract: kernel(**inputs) takes FULL unsharded inputs (as in
reference.setup_inputs()) and returns the FULL [4096, 4, 2048] f32 output.

Strategy (8 cores, data-parallel over tokens, 512 tokens/core):
  - Front: x loaded token-major (4 tiles resident); RMS stats + h_all =
    xn @ W_all.T in f32r (PE transposes, 256-wide token pairs). Only the
    read-in gates ri are computed before the MLP; the rest of the per-token
    generator scalars (4x4 Cayley closed form) runs overlapped with MLP1.
  - branch = sum_n ri_n x_n quantized to fp8-e4m3 hi+lo pairs (quant error
    compensated); W1/W2 pre-scaled x256 and cast to fp8 on the host.
  - MLP entirely fp8 DoubleRow (2 k-planes per matmul instruction): MLP1
    computes branch-hi and branch-lo products; g1 = gelu re-quantized hi+lo;
    MLP2 likewise. Residual error is weight quantization only (~1.6e-2 rel).
  - Stream mix out_i = sum_j Phi_ij x_j computed in bf16 (2x DVE rate) from
    a host-cast bf16 copy of x, overlapped with MLP; the tail is fused into
    MLP2 half 1 per 512-wide embed chunk (transpose y from PSUM, STT with
    wo_i straight out of PSUM, chunked output DMAs).
"""
import numpy as np
import ml_dtypes

import concourse.bass as bass
import concourse.bacc as bacc
import concourse.mybir as mybir
import concourse.tile as tile
from concourse.bass_utils import run_bass_kernel_spmd
from concourse.masks import make_identity
from contextlib import ExitStack

F32 = mybir.dt.float32
F32R = mybir.dt.float32r
BF16 = mybir.dt.bfloat16
F8 = mybir.dt.float8e4
AF = mybir.ActivationFunctionType
OP = mybir.AluOpType
DR = mybir.MatmulPerfMode.DoubleRow

NCORES = 8
B_FULL = 4096
TPC = B_FULL // NCORES          # 512 tokens per core
TT = TPC // 128                 # 4 token tiles
NS = 4                          # streams (N)
ED = 2048                       # EMBED / BLOCK
IN_DIM = NS * ED                # 8192
FD = 8192                       # DFF
KT_IN = IN_DIM // 128           # 64 k-tiles over input dim
KT_ED = ED // 128               # 16 k-tiles over embed
KP_ED = KT_ED // 2              # 8 k-pairs over embed
KT_FDH = 32                     # dff k-tiles per half
KP_FDH = KT_FDH // 2            # 16 k-pairs per dff half
WSCALE = 256.0
DT_MIN, DT_MAX = 1e-3, 1.0
DT_RANGE = DT_MAX - DT_MIN
EPS = 1.1920929e-7

PAIRS = [(0, 1), (0, 2), (0, 3), (2, 3), (1, 3), (1, 2)]
PIDX = {p: k for k, p in enumerate(PAIRS)}
U_ROWS = [4 * i + j for (i, j) in PAIRS]
L_ROWS = [4 * j + i for (i, j) in PAIRS]


def build_nc():
    nc = bacc.Bacc()
    x_d = nc.declare_dram_parameter("x", [TPC, NS, ED], F32, isOutput=False)
    xh_d = nc.declare_dram_parameter("xh", [TPC, NS, ED], BF16, isOutput=False)
    wall_d = nc.declare_dram_parameter("wall", [IN_DIM, 32], F32R, isOutput=False)
    # w1: [mgrp(16), p(128), kp(8) x ktl(2) x m(512)] - one DMA per mgrp
    w1_d = nc.declare_dram_parameter("w1", [16, 128, KP_ED * 2 * 512], F8,
                                     isOutput=False)
    # w2: [half(2), mgrp2(4), kph(2), p(128), kp(8) x ktl(2) x role(2) x m(512)]
    w2_d = nc.declare_dram_parameter("w2", [2, 4, 2, 128, 8 * 2 * 2 * 512], F8,
                                     isOutput=False)
    cvec_d = nc.declare_dram_parameter("cvec", [1, 64], F32, isOutput=False)
    out_d = nc.declare_dram_parameter("out", [TPC, NS, ED], F32, isOutput=True)

    with tile.TileContext(nc) as tc, ExitStack() as S0:
        const = S0.enter_context(tc.tile_pool(name="const", bufs=1))
        scal = S0.enter_context(tc.tile_pool(name="scal", bufs=1))
        soupp = S0.enter_context(tc.tile_pool(name="soupp", bufs=4))
        brp = S0.enter_context(tc.tile_pool(name="brp", bufs=1))

        ident = const.tile([128, 128], F32)
        make_identity(nc, ident[:])
        ones1 = const.tile([1, 128], F32)
        nc.gpsimd.memset(ones1[:], 1.0)
        cvec_sb = const.tile([1, 64], F32)
        nc.sync.dma_start(cvec_sb[:], cvec_d[:])

        # persistent per-token scalar outputs
        C = const.tile([128, 64], F32)
        rms = scal.tile([128, TT], F32)
        hscal = scal.tile([128, 32 * TT], F32)
        ri4 = [scal.tile([128, 4], F32, tag=f"ri{t}", name=f"ri{t}") for t in range(TT)]
        wo2 = [scal.tile([128, 4], F32, tag=f"wo{t}", name=f"wo{t}") for t in range(TT)]
        PhiP = [scal.tile([128, 6], F32, tag=f"pp{t}", name=f"pp{t}") for t in range(TT)]
        PhiM = [scal.tile([128, 6], F32, tag=f"pm{t}", name=f"pm{t}") for t in range(TT)]
        PhiD = [scal.tile([128, 4], F32, tag=f"pd{t}", name=f"pd{t}") for t in range(TT)]

        def phi_ap(t, i, j):
            if i == j:
                return PhiD[t][:, i:i + 1]
            if (i, j) in PIDX:
                return PhiP[t][:, PIDX[(i, j)]:PIDX[(i, j)] + 1]
            return PhiM[t][:, PIDX[(j, i)]:PIDX[(j, i)] + 1]

        # branchT fp8 hi+lo: [128(k), kt(16) x role(2) x tok(512)] = 16KB
        brT = brp.tile([128, KT_ED * 2 * TPC], F8)
        brT_v = brT[:].rearrange("p (kt r n) -> p kt r n", kt=KT_ED, r=2)

        # ================= FRONT =================
        with ExitStack() as SA:
            xtokp = SA.enter_context(tc.tile_pool(name="xtok", bufs=1))
            wallp = SA.enter_context(tc.tile_pool(name="wallp", bufs=1))
            xtp = SA.enter_context(tc.tile_pool(name="xtp", bufs=3))
            wkp = SA.enter_context(tc.tile_pool(name="wk", bufs=2))
            sqp = SA.enter_context(tc.tile_pool(name="sqp", bufs=1))
            brtok = SA.enter_context(tc.tile_pool(name="brtok", bufs=2))
            trps = SA.enter_context(tc.tile_pool(name="trps", bufs=2, space="PSUM"))
            haccp = SA.enter_context(tc.tile_pool(name="haccp", bufs=1, space="PSUM"))

            # broadcast cvec over partitions via PE outer product
            cps = haccp.tile([128, 512], F32, tag="cps")
            nc.tensor.matmul(cps[:, :64], ones1[:], cvec_sb[:])
            nc.vector.tensor_copy(C[:], cps[:, :64])

            wall_sb = wallp.tile([128, KT_IN * 32], F32R)
            nc.sync.dma_start(
                wall_sb[:], wall_d[:].rearrange("(kt p) m -> p kt m", p=128))

            # P1+P2: load x tiles; squares + per-tile rms; transpose token
            # pairs (256-wide) and accumulate h = wall.T @ xT (f32r, free 256).
            # As soon as a pair's h is done: hscal, ri, branch, brT quant for
            # its two tiles, so the fp8 MLP inputs are ready ASAP.
            h_ps = haccp.tile([128, 512], F32, tag="hps")
            ssq4 = scal.tile([128, TT * 4], F32)
            hT_sb = scal.tile([32, 512], F32, tag="hT")
            x_tok = []
            for t in range(TT):
                xt = xtokp.tile([128, NS * ED], F32, tag=f"xt{t}", name=f"xt{t}")
                nc.sync.dma_start(xt[:], x_d[t * 128:(t + 1) * 128])
                x_tok.append(xt)
                for j in range(NS):
                    sc = sqp.tile([128, ED], F32, tag="sq")
                    nc.scalar.activation(
                        sc[:], xt[:, j * ED:(j + 1) * ED], AF.Square,
                        accum_out=ssq4[:, t * 4 + j:t * 4 + j + 1])
                ssq1 = wkp.tile([128, 1], F32, tag="ssq1", name=f"ssq1_{t}")
                nc.vector.reduce_sum(ssq1[:], ssq4[:, t * 4:(t + 1) * 4],
                                     axis=mybir.AxisListType.X)
                vmean = wkp.tile([128, 1], F32, tag="vmean", name=f"vmean{t}")
                nc.scalar.activation(vmean[:], ssq1[:], AF.Copy,
                                     bias=EPS, scale=1.0 / IN_DIM)
                vinv = wkp.tile([128, 1], F32, tag="vinv", name=f"vinv{t}")
                nc.vector.reciprocal(vinv[:], vmean[:])
                nc.scalar.activation(rms[:, t:t + 1], vinv[:], AF.Sqrt)
                if t % 2 == 0:
                    continue
                pair = t // 2
                for fg in range(KT_IN // 4):
                    xt_ps = trps.tile([128, 1024], F32, tag="trp")
                    for tloc in range(2):
                        for kk in range(4):
                            ft = fg * 4 + kk
                            nc.tensor.transpose(
                                xt_ps[:, kk * 256 + tloc * 128:
                                      kk * 256 + (tloc + 1) * 128],
                                x_tok[pair * 2 + tloc][:, ft * 128:(ft + 1) * 128],
                                ident[:])
                    xt_sb = xtp.tile([128, 1024], F32R, tag="xt")
                    (nc.vector if fg % 2 == 0 else nc.gpsimd).tensor_copy(
                        xt_sb[:], xt_ps[:])
                    for kk in range(4):
                        ft = fg * 4 + kk
                        nc.tensor.matmul(
                            h_ps[:32, pair * 256:(pair + 1) * 256],
                            wall_sb[:, ft * 32:(ft + 1) * 32],
                            xt_sb[:, kk * 256:(kk + 1) * 256],
                            start=(ft == 0), stop=(ft == KT_IN - 1),
                            skip_group_check=True)

                # per-pair: h -> token-major (all PE transpose work for the
                # next pair is emitted before the data-dependent chains)
                nc.vector.tensor_copy(hT_sb[:, pair * 256:(pair + 1) * 256],
                                      h_ps[:32, pair * 256:(pair + 1) * 256])

            # per-tile: rms-scaled h, ri gate, branch, brT hi+lo quant
            for tl in range(TT):
                if True:
                    hps2 = trps.tile([128, 512], F32, tag="btp")
                    nc.tensor.transpose(
                        hps2[:, :32], hT_sb[:, tl * 128:(tl + 1) * 128],
                        ident[:32, :32])
                    nc.vector.tensor_scalar_mul(
                        hscal[:, tl * 32:(tl + 1) * 32], hps2[:, :32],
                        rms[:, tl:tl + 1])
                    pr = soupp.tile([128, 4], F32, tag="w4a", name=f"w4a_{tl}")
                    nc.vector.scalar_tensor_tensor(
                        pr[:], hscal[:, tl * 32:tl * 32 + 4], C[:, 8:9],
                        C[:, 0:4], OP.mult, OP.add)
                    nc.scalar.activation(ri4[tl][:], pr[:], AF.Sigmoid)

                    br = brtok.tile([128, ED], F32, tag="br", name=f"br{tl}")
                    e0 = nc.vector if tl % 2 == 0 else nc.gpsimd
                    e0.tensor_scalar_mul(br[:], x_tok[tl][:, 0:ED],
                                         ri4[tl][:, 0:1])
                    for j in range(1, NS):
                        e0.scalar_tensor_tensor(
                            br[:], x_tok[tl][:, j * ED:(j + 1) * ED],
                            ri4[tl][:, j:j + 1], br[:], OP.mult, OP.add)
                    for kg in range(KT_ED // 4):
                        bt_ps = trps.tile([128, 512], F32, tag="btp")
                        for kk in range(4):
                            kt = kg * 4 + kk
                            nc.tensor.transpose(
                                bt_ps[:, kk * 128:(kk + 1) * 128],
                                br[:, kt * 128:(kt + 1) * 128], ident[:])
                        src = bt_ps[:].rearrange("p (four n) -> p four n",
                                                 four=4)
                        hi = brT_v[:, kg * 4:(kg + 1) * 4, 0,
                                   tl * 128:(tl + 1) * 128]
                        lo = brT_v[:, kg * 4:(kg + 1) * 4, 1,
                                   tl * 128:(tl + 1) * 128]
                        e1 = nc.gpsimd if kg % 2 == 0 else nc.vector
                        e1.tensor_copy(hi, src)
                        e1.tensor_sub(lo, src, hi)

        # ================= MLP + SOUP + MIX + TAIL =================
        with ExitStack() as SB:
            mixp = SB.enter_context(tc.tile_pool(name="mixp", bufs=1))
            yaccp = SB.enter_context(tc.tile_pool(name="yaccp", bufs=1))
            g1p = SB.enter_context(tc.tile_pool(name="g1p", bufs=1))
            w2sp = SB.enter_context(tc.tile_pool(name="w2s", bufs=2))
            mlpps = SB.enter_context(tc.tile_pool(name="mlpps", bufs=1, space="PSUM"))
            SE = SB.enter_context(ExitStack())
            xhp = SE.enter_context(tc.tile_pool(name="xhp", bufs=1))
            w1sp = SE.enter_context(tc.tile_pool(name="w1s", bufs=2))

            mix = [mixp.tile([128, NS * ED], BF16, tag=f"mx{t}", name=f"mx{t}")
                   for t in range(TT)]
            yacc = yaccp.tile([128, KT_ED * TPC], F32)
            # g1T single fp8 (gelu writes it directly): [128, kth(32) x 512]
            g1T = g1p.tile([128, KT_FDH * TPC], F8)
            g1T_v = g1T[:].rearrange("p (kt n) -> p kt n", kt=KT_FDH)

            def mlp1_half(half):
                for mgrp in range(half * 8, half * 8 + 8):  # groups of 512 dff
                    g_ps = [mlpps.tile([128, 512], F32,
                                       tag=f"mm{(mgrp * 4 + m) % 6}",
                                       name=f"gps{mgrp}_{m}") for m in range(4)]
                    w1t = w1sp.tile([128, KP_ED * 2 * 512], F8, tag="w1t")
                    nc.sync.dma_start(w1t[:], w1_d[mgrp])
                    w1v = w1t[:].rearrange("p (kp two m) -> p kp two m",
                                           kp=KP_ED, two=2)
                    for kp in range(KP_ED):
                        for m in range(4):
                            lhsT = w1v[:, kp, :, m * 128:(m + 1) * 128]
                            for role in range(2):
                                nc.tensor.matmul(
                                    g_ps[m][:], lhsT,
                                    brT_v[:, 2 * kp:2 * kp + 2, role, :],
                                    start=(kp == 0 and role == 0),
                                    stop=(kp == KP_ED - 1 and role == 1),
                                    perf_mode=DR)
                    for m in range(4):
                        mt = mgrp * 4 + m          # global dff m-tile
                        kth = mt - half * KT_FDH   # k-tile within half
                        nc.scalar.activation(g1T_v[:, kth, :], g_ps[m][:],
                                             AF.Gelu_apprx_tanh,
                                             scale=1.0 / WSCALE)

            def mlp2_grp(half, mgrp2):
                y_ps = [mlpps.tile([128, 512], F32,
                                   tag=f"mm{(mgrp2 * 4 + m) % 6}",
                                   name=f"yps{half}_{mgrp2}_{m}")
                        for m in range(4)]
                w2v = []
                for kph in range(2):
                    w2t = w2sp.tile([128, 8 * 2 * 2 * 512], F8, tag="w2t")
                    nc.sync.dma_start(w2t[:], w2_d[half, mgrp2, kph])
                    w2v.append(w2t[:].rearrange(
                        "p (kp two r m) -> p kp two r m", kp=8, two=2, r=2))
                for kp in range(KP_FDH):
                    for m in range(4):
                        for role in range(2):
                            lhsT = w2v[kp // 8][:, kp % 8, :, role,
                                               m * 128:(m + 1) * 128]
                            nc.tensor.matmul(
                                y_ps[m][:], lhsT,
                                g1T_v[:, 2 * kp:2 * kp + 2, :],
                                start=(kp == 0 and role == 0),
                                stop=(kp == KP_FDH - 1 and role == 1),
                                perf_mode=DR)
                return y_ps

            # ---- MLP1 half 0 (g-quant split DVE/Pool) ----
            mlp1_half(0)

            # ---- rest of the scalar soup (overlaps MLP1 half 0) ----
            for t in range(TT):
                hs = hscal[:, t * 32:(t + 1) * 32]
                w = lambda cols, tg: soupp.tile([128, cols], F32, tag=tg,
                                                name=f"{tg}_{t}")

                pre_wo = w(4, "w4b")
                nc.vector.scalar_tensor_tensor(
                    pre_wo[:], hs[:, 4:8], C[:, 9:10], C[:, 4:8], OP.mult, OP.add)
                nc.scalar.activation(wo2[t][:], pre_wo[:], AF.Sigmoid)
                nc.scalar.mul(wo2[t][:], wo2[t][:], 2.0)

                pre_dt = w(2, "w2a")
                nc.vector.tensor_add(pre_dt[:], hs[:, 8:10], C[:, 10:12])
                sg = w(2, "w2b")
                nc.scalar.activation(sg[:], pre_dt[:], AF.Sigmoid)
                dt2 = w(2, "w2c")
                nc.scalar.activation(dt2[:], sg[:], AF.Copy,
                                     bias=DT_MIN, scale=DT_RANGE)

                pre_d = w(4, "w4c")
                nc.vector.tensor_add(pre_d[:], hs[:, 10:14], C[:, 12:16])
                esp = w(4, "w4f")
                nc.scalar.activation(esp[:], pre_d[:], AF.Exp)
                dsp = w(4, "w4d")
                nc.scalar.activation(dsp[:], esp[:], AF.Ln, bias=1.0)
                dscaled = w(4, "w4e")
                nc.vector.tensor_scalar_mul(dscaled[:], dsp[:], dt2[:, 1:2])
                ehD = w(4, f"ehD{t}")
                nc.scalar.activation(ehD[:], dscaled[:], AF.Exp, scale=-0.5)

                sdiff = w(6, "w6a")
                nc.vector.tensor_sub(sdiff[:], hs[:, 14:20], hs[:, 20:26])
                spre = w(6, "w6b")
                nc.vector.tensor_add(spre[:], sdiff[:], C[:, 16:22])
                s = w(6, f"s{t}")
                nc.vector.tensor_scalar_mul(s[:], spre[:], dt2[:, 0:1])

                sq = w(6, "w6c")
                nc.vector.tensor_mul(sq[:], s[:], s[:])
                p1 = w(1, "p1")
                nc.vector.reduce_sum(p1[:], sq[:], axis=mybir.AxisListType.X)
                prod3 = w(3, "w3a")
                nc.vector.tensor_mul(prod3[:], s[:, 0:3], s[:, 3:6])
                t1 = w(1, "t1")
                nc.vector.tensor_sub(t1[:], prod3[:, 0:1], prod3[:, 1:2])
                Pf = w(1, "Pf")
                nc.vector.tensor_add(Pf[:], t1[:], prod3[:, 2:3])
                q1 = w(1, "q1")
                nc.vector.tensor_mul(q1[:], Pf[:], Pf[:])
                Dm = w(1, "Dm")
                nc.vector.tensor_add(Dm[:], p1[:], q1[:])
                D1 = w(1, "D1")
                nc.scalar.activation(D1[:], Dm[:], AF.Copy, bias=1.0)
                r0 = w(1, "r0")
                nc.vector.reciprocal(r0[:], D1[:])
                t2 = w(1, "t2")
                nc.vector.tensor_mul(t2[:], D1[:], r0[:])
                t3 = w(1, "t3")
                nc.scalar.activation(t3[:], t2[:], AF.Copy, scale=-1.0, bias=2.0)
                invD = w(1, "invD")
                nc.vector.tensor_mul(invD[:], r0[:], t3[:])

                pr1 = w(2, "pr1")
                nc.vector.tensor_mul(pr1[:], s[:, 0:2], s[:, 4:6])
                pr2 = w(4, "pr2")
                nc.vector.tensor_mul(pr2[:], s[:, 0:4], s[:, 2:6])
                pr3 = w(5, "pr3")
                nc.vector.tensor_mul(pr3[:], s[:, 0:5], s[:, 1:6])
                pr4 = w(1, "pr4")
                nc.vector.tensor_mul(pr4[:], s[:, 0:1], s[:, 5:6])

                cE = w(6, "cE")
                g01 = w(1, "g01")
                nc.vector.tensor_add(g01[:], pr1[:, 1:2], pr2[:, 2:3])
                nc.scalar.activation(cE[:, 0:1], g01[:], AF.Copy, scale=-1.0)
                nc.vector.tensor_sub(cE[:, 1:2], pr4[:, 0:1], pr3[:, 2:3])
                nc.vector.tensor_add(cE[:, 2:3], pr1[:, 0:1], pr2[:, 1:2])
                g23 = w(1, "g23")
                nc.vector.tensor_add(g23[:], pr3[:, 1:2], pr3[:, 4:5])
                nc.scalar.activation(cE[:, 3:4], g23[:], AF.Copy, scale=-1.0)
                nc.vector.tensor_sub(cE[:, 4:5], pr2[:, 3:4], pr2[:, 0:1])
                g12 = w(1, "g12")
                nc.vector.tensor_add(g12[:], pr3[:, 0:1], pr3[:, 3:4])
                nc.scalar.activation(cE[:, 5:6], g12[:], AF.Copy, scale=-1.0)

                mdiag = w(4, "mdiag")
                nc.vector.reduce_sum(mdiag[:, 0:1], sq[:, 0:3],
                                     axis=mybir.AxisListType.X)
                m1a = w(1, "m1a")
                nc.vector.reduce_sum(m1a[:], sq[:, 4:6], axis=mybir.AxisListType.X)
                nc.vector.tensor_add(mdiag[:, 1:2], m1a[:], sq[:, 0:1])
                u1 = w(1, "u1")
                nc.vector.tensor_add(u1[:], sq[:, 1:2], sq[:, 3:4])
                nc.vector.tensor_add(mdiag[:, 2:3], u1[:], sq[:, 5:6])
                nc.vector.reduce_sum(mdiag[:, 3:4], sq[:, 2:5],
                                     axis=mybir.AxisListType.X)

                st6 = w(6, "st6")
                nc.vector.tensor_mul(st6[:, 0:3], s[:, 3:6], C[:, 22:25])
                nc.vector.tensor_mul(st6[:, 3:6], s[:, 0:3], C[:, 22:25])
                o6 = w(6, "o6")
                nc.vector.scalar_tensor_tensor(
                    o6[:], st6[:], Pf[:], s[:], OP.mult, OP.add)
                nplus = w(6, "npl")
                nc.vector.tensor_add(nplus[:], cE[:], o6[:])
                nminus = w(6, "nmi")
                nc.vector.tensor_sub(nminus[:], cE[:], o6[:])

                Ppair = w(6, "Ppair")
                for k, (i, j) in enumerate(PAIRS):
                    nc.vector.tensor_mul(
                        Ppair[:, k:k + 1], ehD[:, i:i + 1], ehD[:, j:j + 1])
                nc.vector.tensor_scalar_mul(Ppair[:], Ppair[:], invD[:])
                nc.scalar.mul(Ppair[:], Ppair[:], 2.0)
                nc.vector.tensor_mul(PhiP[t][:], Ppair[:], nplus[:])
                nc.vector.tensor_mul(PhiM[t][:], Ppair[:], nminus[:])

                base = w(1, "base")
                nc.vector.tensor_sub(base[:], p1[:], q1[:])
                base1 = w(1, "base1")
                nc.scalar.activation(base1[:], base[:], AF.Copy, bias=1.0)
                m2n = w(4, "m2n")
                nc.scalar.mul(m2n[:], mdiag[:], -2.0)
                numd = w(4, "numd")
                nc.vector.tensor_scalar_add(numd[:], m2n[:], base1[:])
                e2 = w(4, "e2")
                nc.vector.tensor_mul(e2[:], ehD[:], ehD[:])
                e2i = w(4, "e2i")
                nc.vector.tensor_scalar_mul(e2i[:], e2[:], invD[:])
                nc.vector.tensor_mul(PhiD[t][:], e2i[:], numd[:])

            # ---- MLP2 half 0 -> yacc ----
            for mgrp2 in range(4):
                y_ps = mlp2_grp(0, mgrp2)
                for m in range(4):
                    mt2 = mgrp2 * 4 + m
                    nc.scalar.activation(
                        yacc[:, mt2 * TPC:(mt2 + 1) * TPC], y_ps[m][:],
                        AF.Copy, scale=1.0 / WSCALE)

            # ---- mix on DVE (bf16, 2x rate), overlaps MLP ----
            for t in range(TT):
                xht = xhp.tile([128, NS * ED], BF16, tag="xh", name=f"xh{t}")
                nc.sync.dma_start(xht[:], xh_d[t * 128:(t + 1) * 128])
                for i in range(NS):
                    acc = mix[t][:, i * ED:(i + 1) * ED]
                    nc.vector.tensor_scalar_mul(
                        acc, xht[:, 0:ED], phi_ap(t, i, 0))
                    for j in range(1, NS):
                        nc.vector.scalar_tensor_tensor(
                            acc, xht[:, j * ED:(j + 1) * ED],
                            phi_ap(t, i, j), acc, OP.mult, OP.add)

            # ---- MLP1 half 1 (g-quant on Pool) ----
            mlp1_half(1)
            SE.close()

            # ---- MLP2 half 1 fused with the tail, per embed chunk ----
            with ExitStack() as SC:
                outp = SC.enter_context(tc.tile_pool(name="outp", bufs=2))
                tps = SC.enter_context(tc.tile_pool(name="tps", bufs=2,
                                                    space="PSUM"))
                for mgrp2 in range(4):
                    y_ps = mlp2_grp(1, mgrp2)
                    for m in range(4):
                        mt2 = mgrp2 * 4 + m
                        dst = yacc[:, mt2 * TPC:(mt2 + 1) * TPC]
                        nc.gpsimd.scalar_tensor_tensor(
                            dst, y_ps[m][:], 1.0 / WSCALE, dst, OP.mult, OP.add)
                    # tail for embed chunk mgrp2: transpose y, fuse wo*y + mix
                    for t in range(TT):
                        yt_ps = tps.tile([128, 512], F32, tag="tp",
                                         name=f"ytps{t}_{mgrp2}")
                        for kk in range(4):
                            m2 = mgrp2 * 4 + kk
                            nc.tensor.transpose(
                                yt_ps[:, kk * 128:(kk + 1) * 128],
                                yacc[:, m2 * TPC + t * 128:
                                     m2 * TPC + (t + 1) * 128], ident[:])
                        ot = outp.tile([128, NS * 512], F32, tag="ot",
                                       name=f"ot{t}_{mgrp2}")
                        for i in range(NS):
                            e1 = nc.vector if i % 2 == 0 else nc.gpsimd
                            e1.scalar_tensor_tensor(
                                ot[:, i * 512:(i + 1) * 512], yt_ps[:],
                                wo2[t][:, i:i + 1],
                                mix[t][:, i * ED + mgrp2 * 512:
                                       i * ED + (mgrp2 + 1) * 512],
                                OP.mult, OP.add)
                        nc.sync.dma_start(
                            out_d[t * 128:(t + 1) * 128, :,
                                  mgrp2 * 512:(mgrp2 + 1) * 512], ot[:])

    nc.compile()
    return nc


_NC_CACHE = None


def _get_nc():
    global _NC_CACHE
    if _NC_CACHE is None:
        _NC_CACHE = build_nc()
    return _NC_CACHE


def prep_inputs(inputs):
    """Host-side prep: pack weights/constants for the kernel."""
    f = lambda k: np.ascontiguousarray(np.asarray(inputs[k], np.float32))
    wall = np.zeros((IN_DIM, 32), np.float32)
    wall[:, 0:4] = f("W_ri").T
    wall[:, 4:8] = f("W_wo").T
    wall[:, 8] = f("W_dtc")[0]
    wall[:, 9] = f("W_dtd")[0]
    wall[:, 10:14] = f("W_diss").T
    wall[:, 14:20] = 0.5 * f("W_conv")[U_ROWS].T
    wall[:, 20:26] = 0.5 * f("W_conv")[L_ROWS].T
    A = f("conserv_A")
    cvec = np.zeros((1, 64), np.float32)
    cvec[0, 0:4] = f("read_in_p")[:, 0]
    cvec[0, 4:8] = f("write_out_p")[:, 0]
    cvec[0, 8] = f("alpha_read_in")[0]
    cvec[0, 9] = f("alpha_write_out")[0]
    cvec[0, 10] = f("log_dt_conserv")[0] + f("b_dtc")[0]
    cvec[0, 11] = f("log_dt_diss")[0] + f("b_dtd")[0]
    cvec[0, 12:16] = f("diss_diag")
    cvec[0, 16:22] = [0.5 * (A[i, j] - A[j, i]) for (i, j) in PAIRS]
    cvec[0, 22:25] = [1.0, -1.0, 1.0]

    # W1 [K=2048, M=8192]: k = kp*256 + ktl*128 + p; m = mgrp*512 + mm
    # -> [mgrp(16), p(128), kp(8), ktl(2), mm(512)], x256, fp8
    w1 = (f("W1") * WSCALE).reshape(KP_ED, 2, 128, 16, 512)
    w1 = np.ascontiguousarray(w1.transpose(3, 2, 0, 1, 4))
    w1 = w1.astype(ml_dtypes.float8_e4m3)
    # W2 [K=8192, M=2048] as fp8 hi+lo planes (weight quant error
    # compensated): k = half*4096 + kph*2048 + kp*256 + ktl*128 + p,
    # m = mgrp2*512 + mm
    # -> [half, mgrp2(4), kph(2), p, kp(8), ktl(2), role(2), mm(512)]
    w2s = f("W2") * WSCALE
    w2hi = w2s.astype(ml_dtypes.float8_e4m3)
    w2lo = (w2s - w2hi.astype(np.float32)).astype(ml_dtypes.float8_e4m3)
    w2 = np.stack([w2hi, w2lo], axis=-2)  # [K, 2, M]
    w2 = w2.reshape(2, 2, 8, 2, 128, 2, 4, 512)   # [half,kph,kp,ktl,p,r,mg2,mm]
    w2 = np.ascontiguousarray(w2.transpose(0, 6, 1, 4, 2, 3, 5, 7))
    w2 = w2.astype(ml_dtypes.float8_e4m3)
    x = f("x")
    xh = np.ascontiguousarray(x.astype(ml_dtypes.bfloat16))
    return {
        "wall": np.ascontiguousarray(wall),
        "w1": w1,
        "w2": w2,
        "cvec": cvec,
        "x": x,
        "xh": xh,
    }


def kernel(**inputs) -> np.ndarray:
    prep = prep_inputs(inputs)
    x = prep["x"]
    xh = prep["xh"]
    nc = _get_nc()
    in_maps = []
    for c in range(NCORES):
        in_maps.append({
            "x": np.ascontiguousarray(x[c * TPC:(c + 1) * TPC]),
            "xh": np.ascontiguousarray(xh[c * TPC:(c + 1) * TPC]),
            "wall": prep["wall"],
            "w1": prep["w1"],
            "w2": prep["w2"],
            "cvec": prep["cvec"],
        })
    res = run_bass_kernel_spmd(nc, in_maps, list(range(NCORES)))
    out = np.concatenate([res.results[c]["out"] for c in range(NCORES)], axis=0)
    return out.astype(np.float32)


# revision 30
# speedup vs baseline: 1.0465x; 1.0387x over previous
"""Trainium2 Bass kernel for nn_ContinuousGenHyperConnectionsStrang.

Contract: kernel(**inputs) takes FULL unsharded inputs (as in
reference.setup_inputs()) and returns the FULL [4096, 4, 2048] f32 output.

Strategy (8 cores, data-parallel over tokens, 512 tokens/core):
  - Front: x loaded token-major (4 tiles resident); RMS stats + h_all =
    xn @ W_all.T in f32r (PE transposes, 256-wide token pairs). Only the
    read-in gates ri are computed before the MLP; the rest of the per-token
    generator scalars (4x4 Cayley closed form) runs overlapped with MLP1.
  - branch = sum_n ri_n x_n quantized to fp8-e4m3 hi+lo pairs (quant error
    compensated); W1/W2 pre-scaled x256 and cast to fp8 on the host.
  - MLP entirely fp8 DoubleRow (2 k-planes per matmul instruction): MLP1
    computes branch-hi and branch-lo products; g1 = gelu re-quantized hi+lo;
    MLP2 likewise. Residual error is weight quantization only (~1.6e-2 rel).
  - Stream mix out_i = sum_j Phi_ij x_j computed in bf16 (2x DVE rate) from
    a host-cast bf16 copy of x, overlapped with MLP; the tail is fused into
    MLP2 half 1 per 512-wide embed chunk (transpose y from PSUM, STT with
    wo_i straight out of PSUM, chunked output DMAs).
"""
import numpy as np
import ml_dtypes

import concourse.bass as bass
import concourse.bacc as bacc
import concourse.mybir as mybir
import concourse.tile as tile
from concourse.bass_utils import run_bass_kernel_spmd
from concourse.masks import make_identity
from contextlib import ExitStack

F32 = mybir.dt.float32
F32R = mybir.dt.float32r
BF16 = mybir.dt.bfloat16
F8 = mybir.dt.float8e4
AF = mybir.ActivationFunctionType
OP = mybir.AluOpType
DR = mybir.MatmulPerfMode.DoubleRow

NCORES = 8
B_FULL = 4096
TPC = B_FULL // NCORES          # 512 tokens per core
TT = TPC // 128                 # 4 token tiles
NS = 4                          # streams (N)
ED = 2048                       # EMBED / BLOCK
IN_DIM = NS * ED                # 8192
FD = 8192                       # DFF
KT_IN = IN_DIM // 128           # 64 k-tiles over input dim
KT_ED = ED // 128               # 16 k-tiles over embed
KP_ED = KT_ED // 2              # 8 k-pairs over embed
KT_FDH = 32                     # dff k-tiles per half
KP_FDH = KT_FDH // 2            # 16 k-pairs per dff half
WSCALE = 256.0
DT_MIN, DT_MAX = 1e-3, 1.0
DT_RANGE = DT_MAX - DT_MIN
EPS = 1.1920929e-7

PAIRS = [(0, 1), (0, 2), (0, 3), (2, 3), (1, 3), (1, 2)]
PIDX = {p: k for k, p in enumerate(PAIRS)}
U_ROWS = [4 * i + j for (i, j) in PAIRS]
L_ROWS = [4 * j + i for (i, j) in PAIRS]


def build_nc():
    nc = bacc.Bacc()
    x_d = nc.declare_dram_parameter("x", [TPC, NS, ED], F32, isOutput=False)
    xh_d = nc.declare_dram_parameter("xh", [TPC, NS, ED], BF16, isOutput=False)
    wall_d = nc.declare_dram_parameter("wall", [IN_DIM, 32], F32R, isOutput=False)
    # w1: [mgrp(16), p(128), kp(8) x ktl(2) x m(512)] - one DMA per mgrp
    w1_d = nc.declare_dram_parameter("w1", [16, 128, KP_ED * 2 * 512], F8,
                                     isOutput=False)
    # w2: [half(2), mgrp2(4), kph(2), p(128), kp(8) x ktl(2) x role(2) x m(512)]
    w2_d = nc.declare_dram_parameter("w2", [2, 4, 2, 128, 8 * 2 * 2 * 512], F8,
                                     isOutput=False)
    cvec_d = nc.declare_dram_parameter("cvec", [1, 64], F32, isOutput=False)
    out_d = nc.declare_dram_parameter("out", [TPC, NS, ED], F32, isOutput=True)

    with tile.TileContext(nc) as tc, ExitStack() as S0:
        const = S0.enter_context(tc.tile_pool(name="const", bufs=1))
        scal = S0.enter_context(tc.tile_pool(name="scal", bufs=1))
        soupp = S0.enter_context(tc.tile_pool(name="soupp", bufs=4))
        brp = S0.enter_context(tc.tile_pool(name="brp", bufs=1))
        # w1 stream pool lives outside the front scope so its SBUF never
        # aliases front tiles: the first weight DMAs can run during the front
        w1sp = S0.enter_context(tc.tile_pool(name="w1s", bufs=2))

        ident = const.tile([128, 128], F32)
        make_identity(nc, ident[:])
        identb = const.tile([128, 128], BF16)
        nc.vector.tensor_copy(identb[:], ident[:])
        ones1 = const.tile([1, 128], F32)
        nc.gpsimd.memset(ones1[:], 1.0)
        cvec_sb = const.tile([1, 64], F32)
        nc.sync.dma_start(cvec_sb[:], cvec_d[:])

        # persistent per-token scalar outputs
        C = const.tile([128, 64], F32)
        rms = scal.tile([128, TT], F32)
        hscal = scal.tile([128, 32 * TT], F32)
        ri4 = [scal.tile([128, 4], F32, tag=f"ri{t}", name=f"ri{t}") for t in range(TT)]
        wo2 = [scal.tile([128, 4], F32, tag=f"wo{t}", name=f"wo{t}") for t in range(TT)]
        PhiP = [scal.tile([128, 6], F32, tag=f"pp{t}", name=f"pp{t}") for t in range(TT)]
        PhiM = [scal.tile([128, 6], F32, tag=f"pm{t}", name=f"pm{t}") for t in range(TT)]
        PhiD = [scal.tile([128, 4], F32, tag=f"pd{t}", name=f"pd{t}") for t in range(TT)]

        def phi_ap(t, i, j):
            if i == j:
                return PhiD[t][:, i:i + 1]
            if (i, j) in PIDX:
                return PhiP[t][:, PIDX[(i, j)]:PIDX[(i, j)] + 1]
            return PhiM[t][:, PIDX[(j, i)]:PIDX[(j, i)] + 1]

        # branchT fp8 hi+lo: [128(k), kt(16) x role(2) x tok(512)] = 16KB
        brT = brp.tile([128, KT_ED * 2 * TPC], F8)
        brT_v = brT[:].rearrange("p (kt r n) -> p kt r n", kt=KT_ED, r=2)

        # ================= FRONT =================
        with ExitStack() as SA:
            xtokp = SA.enter_context(tc.tile_pool(name="xtok", bufs=1))
            wallp = SA.enter_context(tc.tile_pool(name="wallp", bufs=1))
            xtp = SA.enter_context(tc.tile_pool(name="xtp", bufs=2))
            wkp = SA.enter_context(tc.tile_pool(name="wk", bufs=2))
            sqp = SA.enter_context(tc.tile_pool(name="sqp", bufs=1))
            brtok = SA.enter_context(tc.tile_pool(name="brtok", bufs=1))
            trps = SA.enter_context(tc.tile_pool(name="trps", bufs=2, space="PSUM"))
            haccp = SA.enter_context(tc.tile_pool(name="haccp", bufs=1, space="PSUM"))

            # broadcast cvec over partitions via PE outer product
            cps = haccp.tile([128, 512], F32, tag="cps")
            nc.tensor.matmul(cps[:, :64], ones1[:], cvec_sb[:])
            nc.vector.tensor_copy(C[:], cps[:, :64])

            wall_sb = wallp.tile([128, KT_IN * 32], F32R)
            nc.sync.dma_start(
                wall_sb[:], wall_d[:].rearrange("(kt p) m -> p kt m", p=128))

            # P1+P2: load x tiles; squares + per-tile rms; transpose token
            # pairs (256-wide) and accumulate h = wall.T @ xT (f32r, free 256).
            # As soon as a pair's h is done: hscal, ri, branch, brT quant for
            # its two tiles, so the fp8 MLP inputs are ready ASAP.
            h_ps = haccp.tile([128, 512], F32, tag="hps")
            ssq4 = scal.tile([128, TT * 4], F32)
            hT_sb = scal.tile([32, 512], F32, tag="hT")
            x_tok = []
            br_tok = []
            for t in range(TT):
                xt = xtokp.tile([128, NS * ED], F32, tag=f"xt{t}", name=f"xt{t}")
                nc.sync.dma_start(xt[:], x_d[t * 128:(t + 1) * 128])
                x_tok.append(xt)
                for j in range(NS):
                    sc = sqp.tile([128, ED], BF16, tag="sq")
                    nc.scalar.activation(
                        sc[:], xt[:, j * ED:(j + 1) * ED], AF.Square,
                        accum_out=ssq4[:, t * 4 + j:t * 4 + j + 1])
                ssq1 = wkp.tile([128, 1], F32, tag="ssq1", name=f"ssq1_{t}")
                nc.vector.reduce_sum(ssq1[:], ssq4[:, t * 4:(t + 1) * 4],
                                     axis=mybir.AxisListType.X)
                vmean = wkp.tile([128, 1], F32, tag="vmean", name=f"vmean{t}")
                nc.scalar.activation(vmean[:], ssq1[:], AF.Copy,
                                     bias=EPS, scale=1.0 / IN_DIM)
                vinv = wkp.tile([128, 1], F32, tag="vinv", name=f"vinv{t}")
                nc.vector.reciprocal(vinv[:], vmean[:])
                nc.scalar.activation(rms[:, t:t + 1], vinv[:], AF.Sqrt)
                if t % 2 == 0:
                    continue
                pair = t // 2
                for fg in range(KT_IN // 4):
                    xt_ps = trps.tile([128, 1024], F32, tag="trp")
                    for tloc in range(2):
                        for kk in range(4):
                            ft = fg * 4 + kk
                            nc.tensor.transpose(
                                xt_ps[:, kk * 256 + tloc * 128:
                                      kk * 256 + (tloc + 1) * 128],
                                x_tok[pair * 2 + tloc][:, ft * 128:(ft + 1) * 128],
                                ident[:])
                    xt_sb = xtp.tile([128, 1024], F32R, tag="xt")
                    (nc.vector if fg % 2 == 0 else nc.gpsimd).tensor_copy(
                        xt_sb[:], xt_ps[:])
                    for kk in range(4):
                        ft = fg * 4 + kk
                        nc.tensor.matmul(
                            h_ps[:32, pair * 256:(pair + 1) * 256],
                            wall_sb[:, ft * 32:(ft + 1) * 32],
                            xt_sb[:, kk * 256:(kk + 1) * 256],
                            start=(ft == 0), stop=(ft == KT_IN - 1),
                            skip_group_check=True)

                # per-pair: h -> token-major, rms-scaled h, ri gate, branch.
                # Emitted here so the Act queue runs sqrt/sigmoid for this
                # pair BEFORE the next pair's squares, and DVE/Pool start the
                # branch chains while the next pair's x still streams in.
                nc.vector.tensor_copy(hT_sb[:, pair * 256:(pair + 1) * 256],
                                      h_ps[:32, pair * 256:(pair + 1) * 256])
                for tl in (t - 1, t):
                    hps2 = trps.tile([128, 1024], F32, tag="trp")
                    nc.tensor.transpose(
                        hps2[:, :32], hT_sb[:, tl * 128:(tl + 1) * 128],
                        ident[:32, :32])
                    nc.vector.tensor_scalar_mul(
                        hscal[:, tl * 32:(tl + 1) * 32], hps2[:, :32],
                        rms[:, tl:tl + 1])
                    pr = soupp.tile([128, 4], F32, tag="w4a", name=f"w4a_{tl}")
                    nc.vector.scalar_tensor_tensor(
                        pr[:], hscal[:, tl * 32:tl * 32 + 4], C[:, 8:9],
                        C[:, 0:4], OP.mult, OP.add)
                    nc.scalar.activation(ri4[tl][:], pr[:], AF.Sigmoid)

                    br = brtok.tile([128, ED], BF16, tag=f"br{tl}",
                                    name=f"br{tl}")
                    e0 = nc.vector if tl % 2 == 0 else nc.gpsimd
                    e0.tensor_scalar_mul(br[:], x_tok[tl][:, 0:ED],
                                         ri4[tl][:, 0:1])
                    for j in range(1, NS):
                        e0.scalar_tensor_tensor(
                            br[:], x_tok[tl][:, j * ED:(j + 1) * ED],
                            ri4[tl][:, j:j + 1], br[:], OP.mult, OP.add)
                    br_tok.append(br)

            # brT transposes + hi/lo quant, after all PE h work (so pair 1's
            # h never queues behind pair 0's data-dependent transposes)
            for tl in range(TT):
                br = br_tok[tl]
                for kg in range(KT_ED // 4):
                    bt_ps = trps.tile([128, 512], BF16, tag="btb")
                    for kk in range(4):
                        kt = kg * 4 + kk
                        nc.tensor.transpose(
                            bt_ps[:, kk * 128:(kk + 1) * 128],
                            br[:, kt * 128:(kt + 1) * 128], identb[:])
                    src = bt_ps[:].rearrange("p (four n) -> p four n",
                                             four=4)
                    hi = brT_v[:, kg * 4:(kg + 1) * 4, 0,
                               tl * 128:(tl + 1) * 128]
                    lo = brT_v[:, kg * 4:(kg + 1) * 4, 1,
                               tl * 128:(tl + 1) * 128]
                    e1 = nc.gpsimd if kg % 2 == 0 else nc.vector
                    e1.tensor_copy(hi, src)
                    e1.tensor_sub(lo, src, hi)

        # ================= MLP + SOUP + MIX + TAIL =================
        with ExitStack() as SB:
            mixp = SB.enter_context(tc.tile_pool(name="mixp", bufs=1))
            yaccp = SB.enter_context(tc.tile_pool(name="yaccp", bufs=1))
            g1p = SB.enter_context(tc.tile_pool(name="g1p", bufs=1))
            w2sp = SB.enter_context(tc.tile_pool(name="w2s", bufs=2))
            mlpps = SB.enter_context(tc.tile_pool(name="mlpps", bufs=1, space="PSUM"))
            SE = SB.enter_context(ExitStack())
            xhp = SE.enter_context(tc.tile_pool(name="xhp", bufs=1))
            w1sp = SE.enter_context(tc.tile_pool(name="w1s", bufs=2))

            mix = [mixp.tile([128, NS * ED], BF16, tag=f"mx{t}", name=f"mx{t}")
                   for t in range(TT)]
            yacc = yaccp.tile([128, KT_ED * TPC], F32)
            # g1T single fp8 (gelu writes it directly): [128, kth(32) x 512]
            g1T = g1p.tile([128, KT_FDH * TPC], F8)
            g1T_v = g1T[:].rearrange("p (kt n) -> p kt n", kt=KT_FDH)

            def mlp1_half(half):
                for mgrp in range(half * 8, half * 8 + 8):  # groups of 512 dff
                    g_ps = [mlpps.tile([128, 512], F32,
                                       tag=f"mm{(mgrp * 4 + m) % 6}",
                                       name=f"gps{mgrp}_{m}") for m in range(4)]
                    w1t = w1sp.tile([128, KP_ED * 2 * 512], F8, tag="w1t")
                    nc.sync.dma_start(w1t[:], w1_d[mgrp])
                    w1v = w1t[:].rearrange("p (kp two m) -> p kp two m",
                                           kp=KP_ED, two=2)
                    for kp in range(KP_ED):
                        for m in range(4):
                            lhsT = w1v[:, kp, :, m * 128:(m + 1) * 128]
                            for role in range(2):
                                nc.tensor.matmul(
                                    g_ps[m][:], lhsT,
                                    brT_v[:, 2 * kp:2 * kp + 2, role, :],
                                    start=(kp == 0 and role == 0),
                                    stop=(kp == KP_ED - 1 and role == 1),
                                    perf_mode=DR)
                    for m in range(4):
                        mt = mgrp * 4 + m          # global dff m-tile
                        kth = mt - half * KT_FDH   # k-tile within half
                        nc.scalar.activation(g1T_v[:, kth, :], g_ps[m][:],
                                             AF.Gelu_apprx_tanh,
                                             scale=1.0 / WSCALE)

            def mlp2_grp(half, mgrp2):
                y_ps = [mlpps.tile([128, 512], F32,
                                   tag=f"mm{(mgrp2 * 4 + m) % 6}",
                                   name=f"yps{half}_{mgrp2}_{m}")
                        for m in range(4)]
                w2v = []
                for kph in range(2):
                    w2t = w2sp.tile([128, 8 * 2 * 2 * 512], F8, tag="w2t")
                    nc.sync.dma_start(w2t[:], w2_d[half, mgrp2, kph])
                    w2v.append(w2t[:].rearrange(
                        "p (kp two r m) -> p kp two r m", kp=8, two=2, r=2))
                for kp in range(KP_FDH):
                    for m in range(4):
                        for role in range(2):
                            lhsT = w2v[kp // 8][:, kp % 8, :, role,
                                               m * 128:(m + 1) * 128]
                            nc.tensor.matmul(
                                y_ps[m][:], lhsT,
                                g1T_v[:, 2 * kp:2 * kp + 2, :],
                                start=(kp == 0 and role == 0),
                                stop=(kp == KP_FDH - 1 and role == 1),
                                perf_mode=DR)
                return y_ps

            # ---- MLP1 half 0 (g-quant split DVE/Pool) ----
            mlp1_half(0)

            # ---- rest of the scalar soup (overlaps MLP1 half 0) ----
            for t in range(TT):
                hs = hscal[:, t * 32:(t + 1) * 32]
                w = lambda cols, tg: soupp.tile([128, cols], F32, tag=tg,
                                                name=f"{tg}_{t}")

                pre_wo = w(4, "w4b")
                nc.vector.scalar_tensor_tensor(
                    pre_wo[:], hs[:, 4:8], C[:, 9:10], C[:, 4:8], OP.mult, OP.add)
                nc.scalar.activation(wo2[t][:], pre_wo[:], AF.Sigmoid)
                nc.scalar.mul(wo2[t][:], wo2[t][:], 2.0)

                pre_dt = w(2, "w2a")
                nc.vector.tensor_add(pre_dt[:], hs[:, 8:10], C[:, 10:12])
                sg = w(2, "w2b")
                nc.scalar.activation(sg[:], pre_dt[:], AF.Sigmoid)
                dt2 = w(2, "w2c")
                nc.scalar.activation(dt2[:], sg[:], AF.Copy,
                                     bias=DT_MIN, scale=DT_RANGE)

                pre_d = w(4, "w4c")
                nc.vector.tensor_add(pre_d[:], hs[:, 10:14], C[:, 12:16])
                esp = w(4, "w4f")
                nc.scalar.activation(esp[:], pre_d[:], AF.Exp)
                dsp = w(4, "w4d")
                nc.scalar.activation(dsp[:], esp[:], AF.Ln, bias=1.0)
                dscaled = w(4, "w4e")
                nc.vector.tensor_scalar_mul(dscaled[:], dsp[:], dt2[:, 1:2])
                ehD = w(4, f"ehD{t}")
                nc.scalar.activation(ehD[:], dscaled[:], AF.Exp, scale=-0.5)

                sdiff = w(6, "w6a")
                nc.vector.tensor_sub(sdiff[:], hs[:, 14:20], hs[:, 20:26])
                spre = w(6, "w6b")
                nc.vector.tensor_add(spre[:], sdiff[:], C[:, 16:22])
                s = w(6, f"s{t}")
                nc.vector.tensor_scalar_mul(s[:], spre[:], dt2[:, 0:1])

                sq = w(6, "w6c")
                nc.vector.tensor_mul(sq[:], s[:], s[:])
                p1 = w(1, "p1")
                nc.vector.reduce_sum(p1[:], sq[:], axis=mybir.AxisListType.X)
                prod3 = w(3, "w3a")
                nc.vector.tensor_mul(prod3[:], s[:, 0:3], s[:, 3:6])
                t1 = w(1, "t1")
                nc.vector.tensor_sub(t1[:], prod3[:, 0:1], prod3[:, 1:2])
                Pf = w(1, "Pf")
                nc.vector.tensor_add(Pf[:], t1[:], prod3[:, 2:3])
                q1 = w(1, "q1")
                nc.vector.tensor_mul(q1[:], Pf[:], Pf[:])
                Dm = w(1, "Dm")
                nc.vector.tensor_add(Dm[:], p1[:], q1[:])
                D1 = w(1, "D1")
                nc.scalar.activation(D1[:], Dm[:], AF.Copy, bias=1.0)
                r0 = w(1, "r0")
                nc.vector.reciprocal(r0[:], D1[:])
                t2 = w(1, "t2")
                nc.vector.tensor_mul(t2[:], D1[:], r0[:])
                t3 = w(1, "t3")
                nc.scalar.activation(t3[:], t2[:], AF.Copy, scale=-1.0, bias=2.0)
                invD = w(1, "invD")
                nc.vector.tensor_mul(invD[:], r0[:], t3[:])

                pr1 = w(2, "pr1")
                nc.vector.tensor_mul(pr1[:], s[:, 0:2], s[:, 4:6])
                pr2 = w(4, "pr2")
                nc.vector.tensor_mul(pr2[:], s[:, 0:4], s[:, 2:6])
                pr3 = w(5, "pr3")
                nc.vector.tensor_mul(pr3[:], s[:, 0:5], s[:, 1:6])
                pr4 = w(1, "pr4")
                nc.vector.tensor_mul(pr4[:], s[:, 0:1], s[:, 5:6])

                cE = w(6, "cE")
                g01 = w(1, "g01")
                nc.vector.tensor_add(g01[:], pr1[:, 1:2], pr2[:, 2:3])
                nc.scalar.activation(cE[:, 0:1], g01[:], AF.Copy, scale=-1.0)
                nc.vector.tensor_sub(cE[:, 1:2], pr4[:, 0:1], pr3[:, 2:3])
                nc.vector.tensor_add(cE[:, 2:3], pr1[:, 0:1], pr2[:, 1:2])
                g23 = w(1, "g23")
                nc.vector.tensor_add(g23[:], pr3[:, 1:2], pr3[:, 4:5])
                nc.scalar.activation(cE[:, 3:4], g23[:], AF.Copy, scale=-1.0)
                nc.vector.tensor_sub(cE[:, 4:5], pr2[:, 3:4], pr2[:, 0:1])
                g12 = w(1, "g12")
                nc.vector.tensor_add(g12[:], pr3[:, 0:1], pr3[:, 3:4])
                nc.scalar.activation(cE[:, 5:6], g12[:], AF.Copy, scale=-1.0)

                mdiag = w(4, "mdiag")
                nc.vector.reduce_sum(mdiag[:, 0:1], sq[:, 0:3],
                                     axis=mybir.AxisListType.X)
                m1a = w(1, "m1a")
                nc.vector.reduce_sum(m1a[:], sq[:, 4:6], axis=mybir.AxisListType.X)
                nc.vector.tensor_add(mdiag[:, 1:2], m1a[:], sq[:, 0:1])
                u1 = w(1, "u1")
                nc.vector.tensor_add(u1[:], sq[:, 1:2], sq[:, 3:4])
                nc.vector.tensor_add(mdiag[:, 2:3], u1[:], sq[:, 5:6])
                nc.vector.reduce_sum(mdiag[:, 3:4], sq[:, 2:5],
                                     axis=mybir.AxisListType.X)

                st6 = w(6, "st6")
                nc.vector.tensor_mul(st6[:, 0:3], s[:, 3:6], C[:, 22:25])
                nc.vector.tensor_mul(st6[:, 3:6], s[:, 0:3], C[:, 22:25])
                o6 = w(6, "o6")
                nc.vector.scalar_tensor_tensor(
                    o6[:], st6[:], Pf[:], s[:], OP.mult, OP.add)
                nplus = w(6, "npl")
                nc.vector.tensor_add(nplus[:], cE[:], o6[:])
                nminus = w(6, "nmi")
                nc.vector.tensor_sub(nminus[:], cE[:], o6[:])

                Ppair = w(6, "Ppair")
                for k, (i, j) in enumerate(PAIRS):
                    nc.vector.tensor_mul(
                        Ppair[:, k:k + 1], ehD[:, i:i + 1], ehD[:, j:j + 1])
                nc.vector.tensor_scalar_mul(Ppair[:], Ppair[:], invD[:])
                nc.scalar.mul(Ppair[:], Ppair[:], 2.0)
                nc.vector.tensor_mul(PhiP[t][:], Ppair[:], nplus[:])
                nc.vector.tensor_mul(PhiM[t][:], Ppair[:], nminus[:])

                base = w(1, "base")
                nc.vector.tensor_sub(base[:], p1[:], q1[:])
                base1 = w(1, "base1")
                nc.scalar.activation(base1[:], base[:], AF.Copy, bias=1.0)
                m2n = w(4, "m2n")
                nc.scalar.mul(m2n[:], mdiag[:], -2.0)
                numd = w(4, "numd")
                nc.vector.tensor_scalar_add(numd[:], m2n[:], base1[:])
                e2 = w(4, "e2")
                nc.vector.tensor_mul(e2[:], ehD[:], ehD[:])
                e2i = w(4, "e2i")
                nc.vector.tensor_scalar_mul(e2i[:], e2[:], invD[:])
                nc.vector.tensor_mul(PhiD[t][:], e2i[:], numd[:])

            # ---- MLP2 half 0 -> yacc ----
            for mgrp2 in range(4):
                y_ps = mlp2_grp(0, mgrp2)
                for m in range(4):
                    mt2 = mgrp2 * 4 + m
                    nc.scalar.activation(
                        yacc[:, mt2 * TPC:(mt2 + 1) * TPC], y_ps[m][:],
                        AF.Copy, scale=1.0 / WSCALE)

            # ---- mix on DVE (bf16, 2x rate), overlaps MLP ----
            for t in range(TT):
                xht = xhp.tile([128, NS * ED], BF16, tag="xh", name=f"xh{t}")
                nc.sync.dma_start(xht[:], xh_d[t * 128:(t + 1) * 128])
                for i in range(NS):
                    acc = mix[t][:, i * ED:(i + 1) * ED]
                    nc.vector.tensor_scalar_mul(
                        acc, xht[:, 0:ED], phi_ap(t, i, 0))
                    for j in range(1, NS):
                        nc.vector.scalar_tensor_tensor(
                            acc, xht[:, j * ED:(j + 1) * ED],
                            phi_ap(t, i, j), acc, OP.mult, OP.add)

            # ---- MLP1 half 1 (g-quant on Pool) ----
            mlp1_half(1)
            SE.close()

            # ---- MLP2 half 1 fused with the tail, per embed chunk ----
            with ExitStack() as SC:
                outp = SC.enter_context(tc.tile_pool(name="outp", bufs=2))
                tps = SC.enter_context(tc.tile_pool(name="tps", bufs=2,
                                                    space="PSUM"))
                for mgrp2 in range(4):
                    y_ps = mlp2_grp(1, mgrp2)
                    for m in range(4):
                        mt2 = mgrp2 * 4 + m
                        dst = yacc[:, mt2 * TPC:(mt2 + 1) * TPC]
                        nc.gpsimd.scalar_tensor_tensor(
                            dst, y_ps[m][:], 1.0 / WSCALE, dst, OP.mult, OP.add)
                    # tail for embed chunk mgrp2: transpose y, fuse wo*y + mix
                    for t in range(TT):
                        yt_ps = tps.tile([128, 512], F32, tag="tp",
                                         name=f"ytps{t}_{mgrp2}")
                        for kk in range(4):
                            m2 = mgrp2 * 4 + kk
                            nc.tensor.transpose(
                                yt_ps[:, kk * 128:(kk + 1) * 128],
                                yacc[:, m2 * TPC + t * 128:
                                     m2 * TPC + (t + 1) * 128], ident[:])
                        ot = outp.tile([128, NS * 512], F32, tag="ot",
                                       name=f"ot{t}_{mgrp2}")
                        for i in range(NS):
                            e1 = nc.vector if i % 2 == 0 else nc.gpsimd
                            e1.scalar_tensor_tensor(
                                ot[:, i * 512:(i + 1) * 512], yt_ps[:],
                                wo2[t][:, i:i + 1],
                                mix[t][:, i * ED + mgrp2 * 512:
                                       i * ED + (mgrp2 + 1) * 512],
                                OP.mult, OP.add)
                        nc.sync.dma_start(
                            out_d[t * 128:(t + 1) * 128, :,
                                  mgrp2 * 512:(mgrp2 + 1) * 512], ot[:])

    nc.compile()
    return nc


_NC_CACHE = None


def _get_nc():
    global _NC_CACHE
    if _NC_CACHE is None:
        _NC_CACHE = build_nc()
    return _NC_CACHE


def prep_inputs(inputs):
    """Host-side prep: pack weights/constants for the kernel."""
    f = lambda k: np.ascontiguousarray(np.asarray(inputs[k], np.float32))
    wall = np.zeros((IN_DIM, 32), np.float32)
    wall[:, 0:4] = f("W_ri").T
    wall[:, 4:8] = f("W_wo").T
    wall[:, 8] = f("W_dtc")[0]
    wall[:, 9] = f("W_dtd")[0]
    wall[:, 10:14] = f("W_diss").T
    wall[:, 14:20] = 0.5 * f("W_conv")[U_ROWS].T
    wall[:, 20:26] = 0.5 * f("W_conv")[L_ROWS].T
    A = f("conserv_A")
    cvec = np.zeros((1, 64), np.float32)
    cvec[0, 0:4] = f("read_in_p")[:, 0]
    cvec[0, 4:8] = f("write_out_p")[:, 0]
    cvec[0, 8] = f("alpha_read_in")[0]
    cvec[0, 9] = f("alpha_write_out")[0]
    cvec[0, 10] = f("log_dt_conserv")[0] + f("b_dtc")[0]
    cvec[0, 11] = f("log_dt_diss")[0] + f("b_dtd")[0]
    cvec[0, 12:16] = f("diss_diag")
    cvec[0, 16:22] = [0.5 * (A[i, j] - A[j, i]) for (i, j) in PAIRS]
    cvec[0, 22:25] = [1.0, -1.0, 1.0]

    # W1 [K=2048, M=8192]: k = kp*256 + ktl*128 + p; m = mgrp*512 + mm
    # -> [mgrp(16), p(128), kp(8), ktl(2), mm(512)], x256, fp8
    w1 = (f("W1") * WSCALE).reshape(KP_ED, 2, 128, 16, 512)
    w1 = np.ascontiguousarray(w1.transpose(3, 2, 0, 1, 4))
    w1 = w1.astype(ml_dtypes.float8_e4m3)
    # W2 [K=8192, M=2048] as fp8 hi+lo planes (weight quant error
    # compensated): k = half*4096 + kph*2048 + kp*256 + ktl*128 + p,
    # m = mgrp2*512 + mm
    # -> [half, mgrp2(4), kph(2), p, kp(8), ktl(2), role(2), mm(512)]
    w2s = f("W2") * WSCALE
    w2hi = w2s.astype(ml_dtypes.float8_e4m3)
    w2lo = (w2s - w2hi.astype(np.float32)).astype(ml_dtypes.float8_e4m3)
    w2 = np.stack([w2hi, w2lo], axis=-2)  # [K, 2, M]
    w2 = w2.reshape(2, 2, 8, 2, 128, 2, 4, 512)   # [half,kph,kp,ktl,p,r,mg2,mm]
    w2 = np.ascontiguousarray(w2.transpose(0, 6, 1, 4, 2, 3, 5, 7))
    w2 = w2.astype(ml_dtypes.float8_e4m3)
    x = f("x")
    xh = np.ascontiguousarray(x.astype(ml_dtypes.bfloat16))
    return {
        "wall": np.ascontiguousarray(wall),
        "w1": w1,
        "w2": w2,
        "cvec": cvec,
        "x": x,
        "xh": xh,
    }


def kernel(**inputs) -> np.ndarray:
    prep = prep_inputs(inputs)
    x = prep["x"]
    xh = prep["xh"]
    nc = _get_nc()
    in_maps = []
    for c in range(NCORES):
        in_maps.append({
            "x": np.ascontiguousarray(x[c * TPC:(c + 1) * TPC]),
            "xh": np.ascontiguousarray(xh[c * TPC:(c + 1) * TPC]),
            "wall": prep["wall"],
            "w1": prep["w1"],
            "w2": prep["w2"],
            "cvec": prep["cvec"],
        })
    res = run_bass_kernel_spmd(nc, in_maps, list(range(NCORES)))
    out = np.concatenate([res.results[c]["out"] for c in range(NCORES)], axis=0)
    return out.astype(np.float32)
